# revision 1
# baseline (speedup 1.0000x reference)
"""DeepseekV2 decoder layer on 8 TRN2 NeuronCores (Bass/Tile).

Sharding: TP over heads (2/core) for q/kv_b/attention/o_proj, kv_a replicated,
TP over INTER (1024/core) for the MLP. Chunked AllReduce after o_proj and
chunked ReduceScatter after down_proj, overlapped with compute.

Internal layout is feature-major ("transposed"): activations live as
[feature, token] so every matmul output feeds the next as `rhs` without any
on-device transpose. RoPE pair-swaps, RMSNorm weight folding, the softmax
scaling, and cos/sin tables are all folded into host-side weight prep.
"""

import numpy as np
import ml_dtypes

import concourse.bass as bass
import concourse.mybir as mybir
import concourse.tile as tile
from concourse import bacc
from concourse.bass_utils import run_bass_kernel_spmd

BF = ml_dtypes.bfloat16

B, S, HID = 2, 1024, 2048
T = B * S                      # 2048 tokens
H = 16
DN, DR = 128, 64
DQK = DN + DR
DV = 128
KVR = 512
INTER = 8192
EPS = 1e-6
ROPE_BASE = 10000.0
SCALING = DQK ** -0.5

NC_N = 8
HPC = H // NC_N                # 2 heads per core
FPC = INTER // NC_N            # 1024 inter per core
P = 128
HCH = HID // P                 # 16 hid chunks
TT = 4                         # token chunks of 512
TW = T // TT                   # 512
KT = S // P                    # 8 k-tiles of 128 per batch
QT = S // TW                   # 2 q-chunks of 512 per batch
NEG = -30000.0

f32 = mybir.dt.float32
bf16 = mybir.dt.bfloat16
ADD = mybir.AluOpType.add
MUL = mybir.AluOpType.mult
AF = mybir.ActivationFunctionType

_CACHE = {}


def _build():
    nc = bacc.Bacc("TRN2", target_bir_lowering=False, debug=False, num_devices=NC_N)
    dp = lambda n, sh, dt: nc.dram_tensor(n, sh, dt, kind="ExternalInput")
    ht32 = dp("ht32", [HID, T], f32)
    htb = dp("htb", [HID, T], bf16)
    wq = dp("wq", [HID, HPC * DQK], bf16)       # [h0n,h1n,h0x1,h0x2,h1x1,h1x2]
    wkva = dp("wkva", [HID, KVR + DR], bf16)
    wkvbn = dp("wkvbn", [KVR, HPC * DN], bf16)
    wkvbv = dp("wkvbv", [KVR, HPC * DV], bf16)
    wo = dp("wo", [HPC * DV, HID], bf16)
    wg = dp("wg", [HID, FPC], bf16)
    wu = dp("wu", [HID, FPC], bf16)
    wd = dp("wd", [FPC, HID], bf16)
    cosf = dp("cosf", [P, T], bf16)
    sinf = dp("sinf", [P, T], bf16)
    masks = dp("masks", [P, 4, TW], f32)
    out = nc.dram_tensor("o", [HID // NC_N, T], f32, kind="ExternalOutput")
    rg = [list(range(NC_N))]

    with tile.TileContext(nc) as tc:
        with tc.tile_pool(name="const", bufs=1) as cpool, \
             tc.tile_pool(name="dram", bufs=1, space="DRAM") as dram:
            ones_col = cpool.tile([P, 1], bf16)
            nc.vector.memset(ones_col[:], 1.0)
            ones_row = cpool.tile([1, P], bf16)
            nc.vector.memset(ones_row[:], 1.0)
            epsb = cpool.tile([1, 1], f32)
            nc.vector.memset(epsb[:], EPS)

            ar_in = [dram.tile([HID, TW], bf16, name=f"ar_in{t}") for t in range(TT)]
            ar_out = [dram.tile([HID, TW], bf16, addr_space="Shared", name=f"ar_out{t}")
                      for t in range(TT)]
            rs_in = [dram.tile([HID, TW], bf16, name=f"rs_in{t}") for t in range(TT)]
            rs_out = [dram.tile([HID // NC_N, TW], bf16, name=f"rs_out{t}")
                      for t in range(TT)]

            # ============ Phase A: projections + attention ============
            with tc.tile_pool(name="akeep", bufs=1) as akeep, \
                 tc.tile_pool(name="awrk", bufs=3) as awrk, \
                 tc.tile_pool(name="arow", bufs=2) as arow, \
                 tc.tile_pool(name="aps", bufs=1, space="PSUM") as aps:

                # survives both sub-phases
                qsb = akeep.tile([P, 3, T], bf16)          # 12K
                lat = akeep.tile([P, KVR // P, T], bf16)   # 16K
                kpe = akeep.tile([DR, T], bf16)            # 4K
                kva = akeep.tile([P, KVR // P, T], bf16)   # 16K
                bc1 = akeep.tile([P, TT, TW], f32)         # 8K
                bc2 = akeep.tile([P, TT, TW], bf16)        # 4K

                # ---- A1: input norm + q/kv_a projections (ht resident) ----
                with tc.tile_pool(name="atmp1", bufs=1) as a1:
                    ht = a1.tile([P, HCH, T], bf16)        # 64K
                    for o in range(HCH):
                        nc.sync.dma_start(ht[:, o, :], htb.ap()[o * P:(o + 1) * P, :])
                    wq_sb = a1.tile([P, HCH, HPC * DQK], bf16)   # 12K
                    for o in range(HCH):
                        nc.sync.dma_start(wq_sb[:, o, :], wq.ap()[o * P:(o + 1) * P, :])
                    wkva_sb = a1.tile([P, HCH, KVR + DR], bf16)  # 18K
                    for o in range(HCH):
                        nc.sync.dma_start(wkva_sb[:, o, :],
                                          wkva.ap()[o * P:(o + 1) * P, :])

                    # input rmsnorm scale r1[t], broadcast to 128 partitions
                    for t in range(TT):
                        ssp = aps.tile([1, TW], f32, tag="ss", bufs=1, name="ssp")
                        for o in range(HCH):
                            sq = awrk.tile([P, TW], bf16, tag="sq", name="sq")
                            nc.scalar.square(sq[:], ht[:, o, t * TW:(t + 1) * TW])
                            nc.tensor.matmul(ssp[:], ones_col[:], sq[:],
                                             start=(o == 0), stop=(o == HCH - 1))
                        srow = arow.tile([1, TW], f32, tag="srow", name="srow")
                        nc.scalar.activation(srow[:], ssp[:], AF.Sqrt,
                                             bias=epsb[:], scale=1.0 / HID)
                        rrow = arow.tile([1, TW], f32, tag="rrow", name="rrow")
                        nc.vector.reciprocal(rrow[:], srow[:])
                        rb = arow.tile([1, TW], bf16, tag="rb", name="rb")
                        nc.vector.tensor_copy(out=rb[:], in_=rrow[:])
                        bcp = aps.tile([P, TW], f32, tag="big", bufs=2, name="bcp")
                        nc.tensor.matmul(bcp[:], ones_row[:], rb[:],
                                         start=True, stop=True)
                        nc.vector.tensor_copy(out=bc1[:, t, :], in_=bcp[:])

                    # q projection (scaled by r1; SCALING folded into wq on host)
                    for f in range(3):
                        for t in range(TT):
                            qp = aps.tile([P, TW], f32, tag="big", bufs=2, name="qp")
                            for o in range(HCH):
                                nc.tensor.matmul(qp[:], wq_sb[:, o, f * P:(f + 1) * P],
                                                 ht[:, o, t * TW:(t + 1) * TW],
                                                 start=(o == 0), stop=(o == HCH - 1))
                            nc.vector.tensor_tensor(qsb[:, f, t * TW:(t + 1) * TW],
                                                    qp[:], bc1[:, t, :], MUL)

                    # kv_a: latent raw (2nd norm is scale-invariant) + k_pe*r1
                    for t in range(TT):
                        ss2p = aps.tile([1, TW], f32, tag="ss2", bufs=1, name="ss2p")
                        for f in range(KVR // P + 1):
                            wid = P if f < KVR // P else DR
                            lp = aps.tile([P, TW], f32, tag="big", bufs=2, name="lp")
                            for o in range(HCH):
                                nc.tensor.matmul(lp[:wid, :],
                                                 wkva_sb[:, o, f * P:f * P + wid],
                                                 ht[:, o, t * TW:(t + 1) * TW],
                                                 start=(o == 0), stop=(o == HCH - 1))
                            if f < KVR // P:
                                nc.vector.tensor_copy(
                                    out=lat[:, f, t * TW:(t + 1) * TW], in_=lp[:])
                                sq2 = awrk.tile([P, TW], bf16, tag="sq", name="sq2")
                                nc.scalar.square(sq2[:], lp[:])
                                nc.tensor.matmul(ss2p[:], ones_col[:], sq2[:],
                                                 start=(f == 0),
                                                 stop=(f == KVR // P - 1))
                            else:
                                nc.vector.tensor_tensor(kpe[:, t * TW:(t + 1) * TW],
                                                        lp[:DR, :], bc1[:DR, t, :],
                                                        MUL)
                        srow2 = arow.tile([1, TW], f32, tag="srow", name="srow2")
                        nc.scalar.activation(srow2[:], ss2p[:], AF.Sqrt,
                                             bias=epsb[:], scale=1.0 / KVR)
                        rrow2 = arow.tile([1, TW], f32, tag="rrow", name="rrow2")
                        nc.vector.reciprocal(rrow2[:], srow2[:])
                        rb2 = arow.tile([1, TW], bf16, tag="rb", name="rb2")
                        nc.vector.tensor_copy(out=rb2[:], in_=rrow2[:])
                        bcp2 = aps.tile([P, TW], f32, tag="big", bufs=2, name="bcp2")
                        nc.tensor.matmul(bcp2[:], ones_row[:], rb2[:],
                                         start=True, stop=True)
                        nc.vector.tensor_copy(out=bc2[:, t, :], in_=bcp2[:])
                        for f in range(KVR // P):
                            nc.vector.tensor_tensor(kva[:, f, t * TW:(t + 1) * TW],
                                                    lat[:, f, t * TW:(t + 1) * TW],
                                                    bc2[:, t, :], MUL)

                # ---- A2: rope, kv_b, attention, o_proj (+AR) ----
                with tc.tile_pool(name="atmp2", bufs=1) as a2:
                    cs = a2.tile([P, T], bf16)
                    nc.sync.dma_start(cs[:], cosf.ap())
                    sn = a2.tile([P, T], bf16)
                    nc.sync.dma_start(sn[:], sinf.ap())
                    msk = a2.tile([P, 4, TW], f32)
                    nc.sync.dma_start(msk[:], masks.ap())
                    wkvbn_sb = a2.tile([P, KVR // P, HPC * DN], bf16)
                    for o in range(KVR // P):
                        nc.sync.dma_start(wkvbn_sb[:, o, :],
                                          wkvbn.ap()[o * P:(o + 1) * P, :])
                    wkvbv_sb = a2.tile([P, KVR // P, HPC * DV], bf16)
                    for o in range(KVR // P):
                        nc.sync.dma_start(wkvbv_sb[:, o, :],
                                          wkvbv.ap()[o * P:(o + 1) * P, :])
                    wo_sb = a2.tile([P, HPC, HID], bf16)
                    for h in range(HPC):
                        nc.sync.dma_start(wo_sb[:, h, :], wo.ap()[h * P:(h + 1) * P, :])

                    # rope: [x1(32); x2(32)] per head; pair-swap via sbuf dma
                    qrope = []
                    for h in range(HPC):
                        src = qsb[:, 2, :]
                        if h == 0:
                            direct = src[0:DR, :]
                        else:
                            dcp = awrk.tile([DR, T], bf16, tag="rope", name="dcp")
                            nc.sync.dma_start(dcp[:], src[DR:2 * DR, :])
                            direct = dcp[:]
                        sw = awrk.tile([DR, T], bf16, tag="rope", name="qsw")
                        nc.sync.dma_start(sw[0:32, :], src[h * DR + 32:h * DR + 64, :])
                        nc.sync.dma_start(sw[32:64, :], src[h * DR:h * DR + 32, :])
                        qr = a2.tile([DR, T], bf16, name=f"qr{h}")
                        tmp = awrk.tile([DR, T], bf16, tag="rope", name="qtmp")
                        nc.vector.tensor_tensor(tmp[:], direct, cs[0:DR, :], MUL)
                        nc.vector.tensor_tensor(qr[:], sw[:], sn[0:DR, :], MUL)
                        nc.vector.tensor_tensor(qr[:], qr[:], tmp[:], ADD)
                        qrope.append(qr)
                    ksw = awrk.tile([DR, T], bf16, tag="rope", name="ksw")
                    nc.sync.dma_start(ksw[0:32, :], kpe[32:64, :])
                    nc.sync.dma_start(ksw[32:64, :], kpe[0:32, :])
                    krope = a2.tile([DR, T], bf16)
                    ktmp = awrk.tile([DR, T], bf16, tag="rope", name="ktmp")
                    nc.vector.tensor_tensor(ktmp[:], kpe[:], cs[0:DR, :], MUL)
                    nc.vector.tensor_tensor(krope[:], ksw[:], sn[0:DR, :], MUL)
                    nc.vector.tensor_tensor(krope[:], krope[:], ktmp[:], ADD)

                    # kv_b: k_nope (transposed out) + v (natural out)
                    knope = a2.tile([P, HPC, T], bf16)
                    for h in range(HPC):
                        for t in range(TT):
                            kp = aps.tile([P, TW], f32, tag="big", bufs=2, name="kp")
                            for c in range(KVR // P):
                                nc.tensor.matmul(kp[:],
                                                 wkvbn_sb[:, c, h * P:(h + 1) * P],
                                                 kva[:, c, t * TW:(t + 1) * TW],
                                                 start=(c == 0),
                                                 stop=(c == KVR // P - 1))
                            nc.vector.tensor_copy(out=knope[:, h, t * TW:(t + 1) * TW],
                                                  in_=kp[:])
                    vnat = a2.tile([P, T // P, HPC * DV], bf16)
                    for to in range(T // P):
                        vp = aps.tile([P, HPC * DV], f32, tag="vp", bufs=1, name="vp")
                        for c in range(KVR // P):
                            nc.tensor.matmul(vp[:], kva[:, c, to * P:(to + 1) * P],
                                             wkvbv_sb[:, c, :],
                                             start=(c == 0), stop=(c == KVR // P - 1))
                        nc.vector.tensor_copy(out=vnat[:, to, :], in_=vp[:])

                    # attention (scores transposed: [k, q]) + o_proj partial + AR
                    attn = a2.tile([P, HPC, T], bf16)
                    for b in range(B):
                        for qt in range(QT):
                            tt = b * QT + qt
                            qc0 = b * S + qt * TW
                            nkt = 4 * qt + 4
                            for h in range(HPC):
                                dnp = aps.tile([1, TW], f32, tag="den", bufs=1, name="dnp")
                                atp = aps.tile([P, TW], f32, tag="att", bufs=2, name="atp")
                                exs = [None] * nkt

                                def consume(kt):
                                    nc.tensor.matmul(dnp[:], ones_col[:], exs[kt][:],
                                                     start=(kt == 0),
                                                     stop=(kt == nkt - 1))
                                    nc.tensor.matmul(atp[:],
                                                     vnat[:, b * KT + kt,
                                                          h * DV:(h + 1) * DV],
                                                     exs[kt][:],
                                                     start=(kt == 0),
                                                     stop=(kt == nkt - 1))

                                for kt in range(nkt):
                                    kc0 = b * S + kt * P
                                    scp = aps.tile([P, TW], f32, tag="big", bufs=2, name="scp")
                                    nc.tensor.matmul(scp[:],
                                                     knope[:, h, kc0:kc0 + P],
                                                     qsb[:, h, qc0:qc0 + TW],
                                                     start=True, stop=False)
                                    nc.tensor.matmul(scp[:],
                                                     krope[:, kc0:kc0 + P],
                                                     qrope[h][:, qc0:qc0 + TW],
                                                     start=False, stop=True)
                                    ex = awrk.tile([P, TW], bf16, tag="ex", bufs=4,
                                                   name="ex")
                                    j = kt - 4 * qt
                                    if j >= 0:
                                        mtmp = awrk.tile([P, TW], f32, tag="mt",
                                                         name="mtmp")
                                        nc.vector.tensor_tensor(mtmp[:], scp[:],
                                                                msk[:, j, :], ADD)
                                        nc.scalar.activation(ex[:], mtmp[:], AF.Exp)
                                    else:
                                        nc.scalar.activation(ex[:], scp[:], AF.Exp)
                                    exs[kt] = ex
                                    if kt >= 2:
                                        consume(kt - 2)
                                consume(max(nkt - 2, 0))
                                if nkt > 1:
                                    consume(nkt - 1)
                                drow = arow.tile([1, TW], bf16, tag="rb", name="drow")
                                with nc.allow_low_precision(reason="softmax denom"):
                                    nc.vector.reciprocal(drow[:], dnp[:])
                                dbp = aps.tile([P, TW], f32, tag="big", bufs=2, name="dbp")
                                nc.tensor.matmul(dbp[:], ones_row[:], drow[:],
                                                 start=True, stop=True)
                                dbc = awrk.tile([P, TW], f32, tag="mt", name="dbc")
                                nc.vector.tensor_copy(out=dbc[:], in_=dbp[:])
                                nc.vector.tensor_tensor(
                                    attn[:, h, qc0:qc0 + TW], atp[:], dbc[:], MUL)
                            # o_proj partial for this token chunk
                            for ho in range(HCH):
                                op = aps.tile([P, TW], f32, tag="big", bufs=2, name="op")
                                for h in range(HPC):
                                    nc.tensor.matmul(op[:],
                                                     wo_sb[:, h, ho * P:(ho + 1) * P],
                                                     attn[:, h, qc0:qc0 + TW],
                                                     start=(h == 0),
                                                     stop=(h == HPC - 1))
                                osb = awrk.tile([P, TW], bf16, tag="ex", bufs=4, name="osb")
                                nc.vector.tensor_copy(out=osb[:], in_=op[:])
                                nc.sync.dma_start(ar_in[tt][ho * P:(ho + 1) * P, :],
                                                  osb[:])
                            nc.gpsimd.collective_compute(
                                "AllReduce", ADD, ins=[ar_in[tt][:].opt()],
                                outs=[ar_out[tt][:].opt()], replica_groups=rg)

            # ============ Phase B: residual + norm + MLP ============
            with tc.tile_pool(name="bbig", bufs=1) as bbig, \
                 tc.tile_pool(name="bwrk", bufs=3) as bwrk, \
                 tc.tile_pool(name="brow", bufs=1) as brow, \
                 tc.tile_pool(name="bps", bufs=1, space="PSUM") as bps:

                wg_sb = bbig.tile([P, HCH, FPC], bf16)       # 32K
                for o in range(HCH):
                    nc.sync.dma_start(wg_sb[:, o, :], wg.ap()[o * P:(o + 1) * P, :])
                wu_sb = bbig.tile([P, HCH, FPC], bf16)       # 32K
                for o in range(HCH):
                    nc.sync.dma_start(wu_sb[:, o, :], wu.ap()[o * P:(o + 1) * P, :])
                wd_sb = bbig.tile([P, FPC // P, HID], bf16)  # 16K
                for o in range(FPC // P):
                    nc.sync.dma_start(wd_sb[:, o, :], wd.ap()[o * P:(o + 1) * P, :])

                for t in range(TT):
                    # x = hidden + attn_out; later scaled in place to x/8
                    x = bbig.tile([P, HCH, TW], bf16, name="x", tag="x", bufs=2)
                    ssp3 = bps.tile([1, TW], f32, tag="ss", bufs=1, name="ssp3")
                    for o in range(HCH):
                        harp = bwrk.tile([P, TW], f32, tag="harp", name="harp")
                        nc.sync.dma_start(
                            harp[:], ht32.ap()[o * P:(o + 1) * P, t * TW:(t + 1) * TW])
                        arsb = bwrk.tile([P, TW], bf16, tag="arsb", bufs=3, name="arsb")
                        nc.sync.dma_start(arsb[:], ar_out[t][o * P:(o + 1) * P, :])
                        nc.vector.tensor_tensor(x[:, o, :], harp[:], arsb[:], ADD)
                        sq3 = bwrk.tile([P, TW], bf16, tag="sq3", bufs=2, name="sq3")
                        nc.scalar.square(sq3[:], x[:, o, :])
                        nc.tensor.matmul(ssp3[:], ones_col[:], sq3[:],
                                         start=(o == 0), stop=(o == HCH - 1))
                    srow3 = brow.tile([1, TW], f32, tag="srow3", name="srow3")
                    nc.scalar.activation(srow3[:], ssp3[:], AF.Sqrt,
                                         bias=epsb[:], scale=1.0 / HID)
                    rrow3 = brow.tile([1, TW], f32, tag="rrow3", name="rrow3")
                    nc.vector.reciprocal(rrow3[:], srow3[:])
                    rb3 = brow.tile([1, TW], bf16, tag="rb3", name="rb3")
                    nc.vector.tensor_copy(out=rb3[:], in_=rrow3[:])
                    bcp3 = bps.tile([P, TW], f32, tag="gu", bufs=4, name="bcp3")
                    nc.tensor.matmul(bcp3[:], ones_row[:], rb3[:], start=True, stop=True)
                    bc3 = bwrk.tile([P, TW], f32, tag="bc3", bufs=1, name="bc3")
                    nc.vector.tensor_copy(out=bc3[:], in_=bcp3[:])
                    h2 = bbig.tile([P, HCH, TW], bf16, name="h2", tag="h2", bufs=2)
                    for o in range(HCH):
                        nc.vector.tensor_tensor(h2[:, o, :], x[:, o, :], bc3[:], MUL)
                        # x -> x/8 in place (folded residual for ReduceScatter)
                        nc.vector.tensor_scalar_mul(x[:, o, :], x[:, o, :], 0.125)

                    # gate/up/silu
                    act = bbig.tile([P, FPC // P, TW], bf16, name="act", tag="act",
                                    bufs=1)
                    for fi in range(FPC // P):
                        gp = bps.tile([P, TW], f32, tag="gu", bufs=4, name="gp")
                        for o in range(HCH):
                            nc.tensor.matmul(gp[:], wg_sb[:, o, fi * P:(fi + 1) * P],
                                             h2[:, o, :],
                                             start=(o == 0), stop=(o == HCH - 1))
                        up = bps.tile([P, TW], f32, tag="gu", bufs=4, name="up")
                        for o in range(HCH):
                            nc.tensor.matmul(up[:], wu_sb[:, o, fi * P:(fi + 1) * P],
                                             h2[:, o, :],
                                             start=(o == 0), stop=(o == HCH - 1))
                        gs = bwrk.tile([P, TW], f32, tag="gs", bufs=2, name="gs")
                        nc.scalar.activation(gs[:], gp[:], AF.Silu)
                        nc.vector.tensor_tensor(act[:, fi, :], up[:], gs[:], MUL)

                    # down projection partial (+x/8) + RS
                    for ho in range(HCH):
                        dpp = bps.tile([P, TW], f32, tag="d", bufs=2, name="dpp")
                        for c in range(FPC // P):
                            nc.tensor.matmul(dpp[:], wd_sb[:, c, ho * P:(ho + 1) * P],
                                             act[:, c, :],
                                             start=(c == 0), stop=(c == FPC // P - 1))
                        dsb = bwrk.tile([P, TW], bf16, tag="dsb", bufs=3, name="dsb")
                        nc.vector.tensor_tensor(dsb[:], dpp[:], x[:, ho, :], ADD)
                        nc.sync.dma_start(rs_in[t][ho * P:(ho + 1) * P, :], dsb[:])
                    nc.gpsimd.collective_compute(
                        "ReduceScatter", ADD, ins=[rs_in[t][:].opt()],
                        outs=[rs_out[t][:].opt()], replica_groups=rg)
                    for o in range(HID // NC_N // P):
                        fin = bwrk.tile([P, TW], bf16, tag="fin", bufs=2, name="fin")
                        nc.sync.dma_start(fin[:], rs_out[t][o * P:(o + 1) * P, :])
                        finf = bwrk.tile([P, TW], f32, tag="finf", bufs=2, name="finf")
                        nc.vector.tensor_copy(out=finf[:], in_=fin[:])
                        nc.sync.dma_start(
                            out.ap()[o * P:(o + 1) * P, t * TW:(t + 1) * TW], finf[:])
    nc.compile()
    return nc


def _prep(hidden_states, positions, w_in_ln, w_q, w_kv_a, w_kv_a_ln,
          w_kv_b, w_o, w_post_ln, w_gate, w_up, w_down):
    hT = np.ascontiguousarray(
        np.asarray(hidden_states, np.float32).reshape(T, HID).T)

    pos = np.asarray(positions).reshape(-1).astype(np.float64)
    inv = ROPE_BASE ** (-np.arange(0, DR, 2, dtype=np.float64) / DR)
    fr = pos[:, None] * inv[None, :]                      # [T, 32]
    c32 = np.cos(fr).T.astype(np.float32)                 # [32, T]
    s32 = np.sin(fr).T.astype(np.float32)
    cosf = np.concatenate([c32] * 4, 0)
    sinf = np.concatenate([-s32, s32, -s32, s32], 0)

    r = np.arange(P)[:, None]
    c = np.arange(TW)[None, :]
    masks = np.stack([np.where(c >= r + j * P, 0.0, NEG) for j in range(4)],
                     1).astype(np.float32)                # [128, 4, 512]

    w_in_ln = np.asarray(w_in_ln, np.float32)
    wqf = (np.asarray(w_q, np.float32) * w_in_ln[:, None] * SCALING
           ).reshape(HID, H, DQK)
    wkvaf = np.asarray(w_kv_a, np.float32) * w_in_ln[:, None]
    kpe_w = wkvaf[:, KVR:]
    wkva_p = np.concatenate([wkvaf[:, :KVR], kpe_w[:, 0::2], kpe_w[:, 1::2]], 1)
    wkvbf = (np.asarray(w_kv_b, np.float32)
             * np.asarray(w_kv_a_ln, np.float32)[:, None]).reshape(KVR, H, DN + DV)
    w_post_ln = np.asarray(w_post_ln, np.float32)
    wgf = np.asarray(w_gate, np.float32) * w_post_ln[:, None]
    wuf = np.asarray(w_up, np.float32) * w_post_ln[:, None]
    wdf = np.asarray(w_down, np.float32)
    wof = np.asarray(w_o, np.float32).reshape(H, DV, HID)

    in_maps = []
    for core in range(NC_N):
        hs = [2 * core, 2 * core + 1]
        nopes = np.concatenate([wqf[:, h, :DN] for h in hs], 1)
        pes = []
        for h in hs:
            pe = wqf[:, h, DN:]
            pes += [pe[:, 0::2], pe[:, 1::2]]
        wq_c = np.concatenate([nopes] + pes, 1)
        in_maps.append({
            "ht32": hT,
            "htb": hT.astype(BF),
            "wq": wq_c.astype(BF),
            "wkva": wkva_p.astype(BF),
            "wkvbn": np.concatenate([wkvbf[:, h, :DN] for h in hs], 1).astype(BF),
            "wkvbv": np.concatenate([wkvbf[:, h, DN:] for h in hs], 1).astype(BF),
            "wo": np.concatenate([wof[h] for h in hs], 0).astype(BF),
            "wg": wgf[:, core * FPC:(core + 1) * FPC].astype(BF),
            "wu": wuf[:, core * FPC:(core + 1) * FPC].astype(BF),
            "wd": wdf[core * FPC:(core + 1) * FPC, :].astype(BF),
            "cosf": cosf.astype(BF),
            "sinf": sinf.astype(BF),
            "masks": masks,
        })
    return in_maps


def kernel(**inputs):
    if "nc" not in _CACHE:
        _CACHE["nc"] = _build()
    nc = _CACHE["nc"]
    in_maps = _prep(**inputs)
    res = run_bass_kernel_spmd(nc, in_maps, core_ids=list(range(NC_N)))
    outT = np.concatenate([res.results[c]["o"] for c in range(NC_N)], 0)
    return np.ascontiguousarray(outT.T).reshape(B, S, HID).astype(np.float32)



# revision 4
# speedup vs baseline: 1.2667x; 1.2667x over previous
"""DeepseekV2 decoder layer on 8 TRN2 NeuronCores (Bass/Tile).

Sharding: TP over heads (2/core) for q/kv_b/attention/o_proj, kv_a sharded
over tokens (256/core) + AllGather, TP over INTER (1024/core) for the MLP.
Chunked AllReduce after o_proj and chunked ReduceScatter after down_proj,
overlapped with compute.

Internal layout is feature-major ("transposed"): activations live as
[feature, token] so every matmul output feeds the next as `rhs` without any
on-device transpose. RoPE pair-swaps, RMSNorm weight folding, the softmax
scaling, and cos/sin tables are all folded into host-side weight prep.

All DRAM tensors are pre-tiled on the host to [128, ...] partition-major
layout so every load/store is a single large dma_start (128 fat
descriptors) instead of hundreds of small ones.
"""

import numpy as np
import ml_dtypes

import concourse.bass as bass
import concourse.mybir as mybir
import concourse.tile as tile
from concourse import bacc
from concourse.bass_utils import run_bass_kernel_spmd

BF = ml_dtypes.bfloat16

B, S, HID = 2, 1024, 2048
T = B * S                      # 2048 tokens
H = 16
DN, DR = 128, 64
DQK = DN + DR
DV = 128
KVR = 512
INTER = 8192
EPS = 1e-6
ROPE_BASE = 10000.0
SCALING = DQK ** -0.5

NC_N = 8
HPC = H // NC_N                # 2 heads per core
FPC = INTER // NC_N            # 1024 inter per core
P = 128
HCH = HID // P                 # 16 hid chunks
TT = 4                         # token chunks of 512
TW = T // TT                   # 512
TO = T // NC_N                 # 256 own tokens for kv_a
KT = S // P                    # 8 k-tiles of 128 per batch
QT = S // TW                   # 2 q-chunks of 512 per batch
KC = KVR // P                  # 4 kv-lora chunks
NEG = -30000.0

f32 = mybir.dt.float32
bf16 = mybir.dt.bfloat16
ADD = mybir.AluOpType.add
MUL = mybir.AluOpType.mult
BYP = mybir.AluOpType.bypass
AF = mybir.ActivationFunctionType

_CACHE = {}


def _build():
    nc = bacc.Bacc("TRN2", target_bir_lowering=False, debug=False, num_devices=NC_N)
    dp = lambda n, sh, dt: nc.dram_tensor(n, sh, dt, kind="ExternalInput")
    htb = dp("htb", [P, TT, HCH, TW], bf16)     # hidden^T, chunk-tiled
    hto = dp("hto", [P, HCH, TO], bf16)         # own-token slice of hidden^T
    wq = dp("wq", [P, HCH, HPC * DQK], bf16)    # [h0n,h1n,h0x1,h0x2,h1x1,h1x2]
    wkva = dp("wkva", [P, HCH, KVR + 2 * DR], bf16)  # kv cols + pe dup'd twice
    wkvb = dp("wkvb", [P, KC, HPC * (DN + DV)], bf16)
    wo = dp("wo", [P, HPC, HID], bf16)
    wg = dp("wg", [P, HCH, FPC], bf16)
    wu = dp("wu", [P, HCH, FPC], bf16)
    wd = dp("wd", [P, FPC // P, HID], bf16)
    cosf = dp("cosf", [P, T], bf16)
    sinf = dp("sinf", [P, T], bf16)
    masks = dp("masks", [P, 4, TW], f32)
    out = nc.dram_tensor("o", [16, TT * HCH * TW], bf16, kind="ExternalOutput")
    rg = [list(range(NC_N))]

    with tile.TileContext(nc) as tc:
        with tc.tile_pool(name="const", bufs=1) as cpool, \
             tc.tile_pool(name="dram", bufs=1, space="DRAM") as dram, \
             tc.tile_pool(name="mlpw", bufs=1) as mlpw:
            ones_col = cpool.tile([P, 1], bf16)
            nc.vector.memset(ones_col[:], 1.0)
            ones_row = cpool.tile([1, P], bf16)
            nc.vector.memset(ones_row[:], 1.0)
            epsb = cpool.tile([1, 1], f32)
            nc.vector.memset(epsb[:], EPS)

            ag_in = dram.tile([P, KC * TO], bf16, name="ag_in")
            ag_out = dram.tile([NC_N * P, KC * TO], bf16, addr_space="Shared",
                               name="ag_out")
            ar_in = [dram.tile([P, HCH, TW], bf16, name=f"ar_in{t}")
                     for t in range(TT)]
            ar_out = [dram.tile([P, HCH, TW], bf16, addr_space="Shared",
                                name=f"ar_out{t}") for t in range(TT)]
            rs_in = [dram.tile([P, HCH, TW], bf16, name=f"rs_in{t}")
                     for t in range(TT)]
            rs_out = [dram.tile([16, HCH * TW], bf16, name=f"rs_out{t}")
                      for t in range(TT)]

            # ============ Phase A: projections + attention ============
            with tc.tile_pool(name="akeep", bufs=1) as akeep, \
                 tc.tile_pool(name="awrk", bufs=2) as awrk, \
                 tc.tile_pool(name="arow", bufs=2) as arow, \
                 tc.tile_pool(name="aps", bufs=1, space="PSUM") as aps:

                # survives A1 -> A2
                qsb = akeep.tile([P, 3, T], bf16)          # 12K
                kpe2 = akeep.tile([P, T], bf16)            # 4K (dup'd rope rows)

                # ---- A1: input norm + q/kv_a projections ----
                with tc.tile_pool(name="a1", bufs=1) as a1:
                    hto_sb = a1.tile([P, HCH, TO], bf16)
                    nc.sync.dma_start(hto_sb[:], hto.ap())
                    wkva_sb = a1.tile([P, HCH, KVR + 2 * DR], bf16)
                    nc.sync.dma_start(wkva_sb[:], wkva.ap())
                    wq_sb = a1.tile([P, HCH, HPC * DQK], bf16)
                    nc.sync.dma_start(wq_sb[:], wq.ap())

                    # -- kv_a for OWN 256 tokens (sharded), then AllGather --
                    lat_own = a1.tile([P, KC, TO], bf16)
                    ss2p = aps.tile([1, TO], f32, tag="ss2", bufs=1, name="ss2p")
                    for f in range(KC):
                        lp = aps.tile([P, TO], f32, tag="big", bufs=2, name="lp")
                        for o in range(HCH):
                            nc.tensor.matmul(lp[:], wkva_sb[:, o, f * P:(f + 1) * P],
                                             hto_sb[:, o, :],
                                             start=(o == 0), stop=(o == HCH - 1))
                        nc.vector.tensor_copy(out=lat_own[:, f, :], in_=lp[:])
                        sq2 = awrk.tile([P, TO], bf16, tag="sq", name="sq2")
                        nc.scalar.square(sq2[:], lp[:])
                        nc.tensor.matmul(ss2p[:], ones_col[:], sq2[:],
                                         start=(f == 0), stop=(f == KC - 1))
                    srow2 = arow.tile([1, TO], f32, tag="srow", name="srow2")
                    nc.scalar.activation(srow2[:], ss2p[:], AF.Sqrt,
                                         bias=epsb[:], scale=1.0 / KVR)
                    rrow2 = arow.tile([1, TO], f32, tag="rrow", name="rrow2")
                    nc.vector.reciprocal(rrow2[:], srow2[:])
                    rb2 = arow.tile([1, TO], bf16, tag="rb", name="rb2")
                    nc.vector.tensor_copy(out=rb2[:], in_=rrow2[:])
                    bcp2 = aps.tile([P, TO], f32, tag="vp", bufs=1, name="bcp2")
                    nc.tensor.matmul(bcp2[:], ones_row[:], rb2[:],
                                     start=True, stop=True)
                    bc2 = a1.tile([P, TO], f32, name="bc2")
                    nc.vector.tensor_copy(out=bc2[:], in_=bcp2[:])
                    agblk = a1.tile([P, KC, TO], bf16, name="agblk")
                    for f in range(KC):
                        nc.vector.tensor_tensor(agblk[:, f, :], lat_own[:, f, :],
                                                bc2[:], MUL)
                    nc.sync.dma_start(ag_in[:], agblk[:])
                    nc.gpsimd.collective_compute(
                        "AllGather", BYP, ins=[ag_in[:].opt()],
                        outs=[ag_out[:].opt()], replica_groups=rg)

                    # -- input rmsnorm scale + q proj + k_pe, per token chunk --
                    for t in range(TT):
                        ht_t = a1.tile([P, HCH, TW], bf16, tag="ht", bufs=2,
                                       name="ht_t")
                        nc.sync.dma_start(ht_t[:], htb.ap()[:, t, :, :])
                        ssp = aps.tile([1, TW], f32, tag="ss", bufs=1, name="ssp")
                        for o in range(HCH):
                            sq = awrk.tile([P, TW], bf16, tag="sq", name="sq")
                            nc.scalar.square(sq[:], ht_t[:, o, :])
                            nc.tensor.matmul(ssp[:], ones_col[:], sq[:],
                                             start=(o == 0), stop=(o == HCH - 1))
                        srow = arow.tile([1, TW], f32, tag="srow", name="srow")
                        nc.scalar.activation(srow[:], ssp[:], AF.Sqrt,
                                             bias=epsb[:], scale=1.0 / HID)
                        rrow = arow.tile([1, TW], f32, tag="rrow", name="rrow")
                        nc.vector.reciprocal(rrow[:], srow[:])
                        rb = arow.tile([1, TW], bf16, tag="rb", name="rb")
                        nc.vector.tensor_copy(out=rb[:], in_=rrow[:])
                        bcp = aps.tile([P, TW], f32, tag="big", bufs=2, name="bcp")
                        nc.tensor.matmul(bcp[:], ones_row[:], rb[:],
                                         start=True, stop=True)
                        bc1 = a1.tile([P, TW], f32, tag="bc1", bufs=2, name="bc1")
                        nc.vector.tensor_copy(out=bc1[:], in_=bcp[:])

                        # q projection (scaled by r1; SCALING folded into wq)
                        for f in range(3):
                            qp = aps.tile([P, TW], f32, tag="big", bufs=2, name="qp")
                            for o in range(HCH):
                                nc.tensor.matmul(qp[:], wq_sb[:, o, f * P:(f + 1) * P],
                                                 ht_t[:, o, :],
                                                 start=(o == 0), stop=(o == HCH - 1))
                            nc.vector.tensor_tensor(qsb[:, f, t * TW:(t + 1) * TW],
                                                    qp[:], bc1[:], MUL)
                        # k_pe (duplicated rows for both attention heads) * r1
                        kp2 = aps.tile([P, TW], f32, tag="big", bufs=2, name="kp2")
                        for o in range(HCH):
                            nc.tensor.matmul(kp2[:], wkva_sb[:, o, KVR:KVR + 2 * DR],
                                             ht_t[:, o, :],
                                             start=(o == 0), stop=(o == HCH - 1))
                        nc.vector.tensor_tensor(kpe2[:, t * TW:(t + 1) * TW],
                                                kp2[:], bc1[:], MUL)

                # ---- A2: rope, kv_b, attention, o_proj (+AR) ----
                with tc.tile_pool(name="a2", bufs=1) as a2:
                    # prefetch the big MLP weights early (consumed in phase B)
                    wg_sb = mlpw.tile([P, HCH, FPC], bf16)       # 32K
                    nc.sync.dma_start(wg_sb[:], wg.ap())
                    wu_sb = mlpw.tile([P, HCH, FPC], bf16)       # 32K
                    nc.sync.dma_start(wu_sb[:], wu.ap())

                    kva2 = a2.tile([P, NC_N, KC, TO], bf16)      # 16K
                    for r in range(NC_N):
                        nc.sync.dma_start(kva2[:, r, :, :],
                                          ag_out[r * P:(r + 1) * P, :])
                    wkvb_sb = a2.tile([P, KC, HPC * (DN + DV)], bf16)
                    nc.sync.dma_start(wkvb_sb[:], wkvb.ap())
                    wo_sb = a2.tile([P, HPC, HID], bf16)
                    nc.sync.dma_start(wo_sb[:], wo.ap())
                    cs = a2.tile([P, T], bf16)
                    nc.sync.dma_start(cs[:], cosf.ap())
                    sn = a2.tile([P, T], bf16)
                    nc.sync.dma_start(sn[:], sinf.ap())
                    msk = a2.tile([P, 4, TW], f32)
                    nc.sync.dma_start(msk[:], masks.ap())

                    # rope in place: qsb[:,2,:] rows are [h0x1,h0x2,h1x1,h1x2],
                    # kpe2 rows are [x1,x2,x1,x2]; cs=[c,c,c,c], sn=[-s,s,-s,s]
                    for src in (qsb[:, 2, :], kpe2[:]):
                        swp = a2.tile([P, T], bf16, tag="swp", bufs=2, name="swp")
                        for g in range(4):
                            half = 32 if g % 2 == 0 else -32
                            nc.sync.dma_start(swp[g * 32:(g + 1) * 32, :],
                                              src[g * 32 + half:(g + 1) * 32 + half, :])
                        rtmp = a2.tile([P, T], bf16, tag="rtmp", bufs=2, name="rtmp")
                        nc.vector.tensor_tensor(rtmp[:], src, cs[:], MUL)
                        nc.vector.tensor_tensor(src, swp[:], sn[:], MUL)
                        nc.vector.tensor_tensor(src, src, rtmp[:], ADD)

                    # kv_b: k_nope (transposed out) + v (natural out)
                    knope = a2.tile([P, HPC, T], bf16)
                    for h in range(HPC):
                        for t2 in range(NC_N):
                            kp = aps.tile([P, TO], f32, tag="big", bufs=2, name="kp")
                            for c in range(KC):
                                nc.tensor.matmul(kp[:],
                                                 wkvb_sb[:, c, h * P:(h + 1) * P],
                                                 kva2[:, t2, c, :],
                                                 start=(c == 0), stop=(c == KC - 1))
                            nc.vector.tensor_copy(
                                out=knope[:, h, t2 * TO:(t2 + 1) * TO], in_=kp[:])
                    vnat = a2.tile([P, T // P, HPC * DV], bf16)
                    for to in range(T // P):
                        vp = aps.tile([P, HPC * DV], f32, tag="vp", bufs=1, name="vp")
                        for c in range(KC):
                            nc.tensor.matmul(vp[:],
                                             kva2[:, to // 2, c,
                                                  (to % 2) * P:(to % 2 + 1) * P],
                                             wkvb_sb[:, c, HPC * DN:],
                                             start=(c == 0), stop=(c == KC - 1))
                        nc.vector.tensor_copy(out=vnat[:, to, :], in_=vp[:])

                    # attention (scores transposed: [k, q]) + o_proj partial + AR
                    for b in range(B):
                        for qt in range(QT):
                            tt = b * QT + qt
                            qc0 = b * S + qt * TW
                            nkt = 4 * qt + 4
                            attn_t = a2.tile([P, HPC, TW], bf16, tag="attn",
                                             bufs=2, name="attn_t")
                            for h in range(HPC):
                                dnp = aps.tile([1, TW], f32, tag="den", bufs=1,
                                               name="dnp")
                                atp = aps.tile([P, TW], f32, tag="att", bufs=2,
                                               name="atp")
                                exs = [None] * nkt

                                def consume(kt):
                                    nc.tensor.matmul(dnp[:], ones_col[:], exs[kt][:],
                                                     start=(kt == 0),
                                                     stop=(kt == nkt - 1))
                                    nc.tensor.matmul(atp[:],
                                                     vnat[:, b * KT + kt,
                                                          h * DV:(h + 1) * DV],
                                                     exs[kt][:],
                                                     start=(kt == 0),
                                                     stop=(kt == nkt - 1))

                                for kt in range(nkt):
                                    kc0 = b * S + kt * P
                                    scp = aps.tile([P, TW], f32, tag="big", bufs=2,
                                                   name="scp")
                                    nc.tensor.matmul(scp[:],
                                                     knope[:, h, kc0:kc0 + P],
                                                     qsb[:, h, qc0:qc0 + TW],
                                                     start=True, stop=False)
                                    nc.tensor.matmul(
                                        scp[:],
                                        kpe2[h * DR:(h + 1) * DR, kc0:kc0 + P],
                                        qsb[h * DR:(h + 1) * DR, 2, qc0:qc0 + TW],
                                        start=False, stop=True)
                                    ex = awrk.tile([P, TW], bf16, tag="ex", bufs=4,
                                                   name="ex")
                                    j = kt - 4 * qt
                                    if j >= 0:
                                        mtmp = awrk.tile([P, TW], f32, tag="mt",
                                                         name="mtmp")
                                        nc.vector.tensor_tensor(mtmp[:], scp[:],
                                                                msk[:, j, :], ADD)
                                        nc.scalar.activation(ex[:], mtmp[:], AF.Exp)
                                    else:
                                        nc.scalar.activation(ex[:], scp[:], AF.Exp)
                                    exs[kt] = ex
                                    if kt >= 2:
                                        consume(kt - 2)
                                consume(max(nkt - 2, 0))
                                if nkt > 1:
                                    consume(nkt - 1)
                                drow = arow.tile([1, TW], bf16, tag="rb", name="drow")
                                with nc.allow_low_precision(reason="softmax denom"):
                                    nc.vector.reciprocal(drow[:], dnp[:])
                                dbp = aps.tile([P, TW], f32, tag="big", bufs=2,
                                               name="dbp")
                                nc.tensor.matmul(dbp[:], ones_row[:], drow[:],
                                                 start=True, stop=True)
                                dbc = awrk.tile([P, TW], f32, tag="mt", name="dbc")
                                nc.vector.tensor_copy(out=dbc[:], in_=dbp[:])
                                nc.vector.tensor_tensor(
                                    attn_t[:, h, :], atp[:], dbc[:], MUL)
                            # o_proj partial for this token chunk
                            oall = a2.tile([P, HCH, TW], bf16, tag="oall", bufs=1,
                                           name="oall")
                            for ho in range(HCH):
                                op = aps.tile([P, TW], f32, tag="big", bufs=2,
                                              name="op")
                                for h in range(HPC):
                                    nc.tensor.matmul(op[:],
                                                     wo_sb[:, h, ho * P:(ho + 1) * P],
                                                     attn_t[:, h, :],
                                                     start=(h == 0),
                                                     stop=(h == HPC - 1))
                                nc.vector.tensor_copy(out=oall[:, ho, :], in_=op[:])
                            nc.sync.dma_start(ar_in[tt][:], oall[:])
                            nc.gpsimd.collective_compute(
                                "AllReduce", ADD, ins=[ar_in[tt][:].opt()],
                                outs=[ar_out[tt][:].opt()], replica_groups=rg)

            # ============ Phase B: residual + norm + MLP ============
            with tc.tile_pool(name="bbig", bufs=1) as bbig, \
                 tc.tile_pool(name="bwrk", bufs=2) as bwrk, \
                 tc.tile_pool(name="brow", bufs=1) as brow, \
                 tc.tile_pool(name="bps", bufs=1, space="PSUM") as bps:

                wd_sb = bbig.tile([P, FPC // P, HID], bf16)  # 32K
                nc.sync.dma_start(wd_sb[:], wd.ap())

                for t in range(TT):
                    # x = hidden + attn_out; later scaled in place to x/8
                    x = bbig.tile([P, HCH, TW], bf16, name="x", tag="x", bufs=2)
                    nc.sync.dma_start(x[:], htb.ap()[:, t, :, :])
                    arall = bbig.tile([P, HCH, TW], bf16, name="arall", tag="ar",
                                      bufs=1)
                    nc.sync.dma_start(arall[:], ar_out[t][:])
                    ssp3 = bps.tile([1, TW], f32, tag="ss", bufs=1, name="ssp3")
                    for o in range(HCH):
                        nc.vector.tensor_tensor(x[:, o, :], x[:, o, :],
                                                arall[:, o, :], ADD)
                        sq3 = bwrk.tile([P, TW], bf16, tag="sq3", bufs=2, name="sq3")
                        nc.scalar.square(sq3[:], x[:, o, :])
                        nc.tensor.matmul(ssp3[:], ones_col[:], sq3[:],
                                         start=(o == 0), stop=(o == HCH - 1))
                    srow3 = brow.tile([1, TW], f32, tag="srow3", name="srow3")
                    nc.scalar.activation(srow3[:], ssp3[:], AF.Sqrt,
                                         bias=epsb[:], scale=1.0 / HID)
                    rrow3 = brow.tile([1, TW], f32, tag="rrow3", name="rrow3")
                    nc.vector.reciprocal(rrow3[:], srow3[:])
                    rb3 = brow.tile([1, TW], bf16, tag="rb3", name="rb3")
                    nc.vector.tensor_copy(out=rb3[:], in_=rrow3[:])
                    bcp3 = bps.tile([P, TW], f32, tag="gu", bufs=4, name="bcp3")
                    nc.tensor.matmul(bcp3[:], ones_row[:], rb3[:], start=True,
                                     stop=True)
                    bc3 = bwrk.tile([P, TW], f32, tag="bc3", bufs=1, name="bc3")
                    nc.vector.tensor_copy(out=bc3[:], in_=bcp3[:])
                    h2 = bbig.tile([P, HCH, TW], bf16, name="h2", tag="h2", bufs=1)
                    for o in range(HCH):
                        nc.vector.tensor_tensor(h2[:, o, :], x[:, o, :], bc3[:], MUL)
                    # x -> x/8 in place (folded residual for ReduceScatter)
                    nc.vector.tensor_scalar_mul(x[:, :, :], x[:, :, :], 0.125)

                    # gate/up/silu
                    act = bbig.tile([P, FPC // P, TW], bf16, name="act", tag="act",
                                    bufs=1)
                    for fi in range(FPC // P):
                        gp = bps.tile([P, TW], f32, tag="gu", bufs=4, name="gp")
                        for o in range(HCH):
                            nc.tensor.matmul(gp[:], wg_sb[:, o, fi * P:(fi + 1) * P],
                                             h2[:, o, :],
                                             start=(o == 0), stop=(o == HCH - 1))
                        up = bps.tile([P, TW], f32, tag="gu", bufs=4, name="up")
                        for o in range(HCH):
                            nc.tensor.matmul(up[:], wu_sb[:, o, fi * P:(fi + 1) * P],
                                             h2[:, o, :],
                                             start=(o == 0), stop=(o == HCH - 1))
                        gs = bwrk.tile([P, TW], f32, tag="gs", bufs=2, name="gs")
                        nc.scalar.activation(gs[:], gp[:], AF.Silu)
                        nc.vector.tensor_tensor(act[:, fi, :], up[:], gs[:], MUL)

                    # down projection partial (+x/8) + RS
                    dall = bbig.tile([P, HCH, TW], bf16, name="dall", tag="dall",
                                     bufs=1)
                    for ho in range(HCH):
                        dpp = bps.tile([P, TW], f32, tag="d", bufs=2, name="dpp")
                        for c in range(FPC // P):
                            nc.tensor.matmul(dpp[:], wd_sb[:, c, ho * P:(ho + 1) * P],
                                             act[:, c, :],
                                             start=(c == 0), stop=(c == FPC // P - 1))
                        nc.vector.tensor_tensor(dall[:, ho, :], dpp[:], x[:, ho, :],
                                                ADD)
                    nc.sync.dma_start(rs_in[t][:], dall[:])
                    nc.gpsimd.collective_compute(
                        "ReduceScatter", ADD, ins=[rs_in[t][:].opt()],
                        outs=[rs_out[t][:].opt()], replica_groups=rg)
                    nc.sync.dma_start(
                        out.ap()[:, t * HCH * TW:(t + 1) * HCH * TW],
                        rs_out[t][:])
    nc.compile()
    return nc


def _prep(hidden_states, positions, w_in_ln, w_q, w_kv_a, w_kv_a_ln,
          w_kv_b, w_o, w_post_ln, w_gate, w_up, w_down):
    hT = np.ascontiguousarray(
        np.asarray(hidden_states, np.float32).reshape(T, HID).T)
    hTt = hT.reshape(HCH, P, T).transpose(1, 0, 2)          # [128, 16, T]
    htb = np.ascontiguousarray(
        hTt.reshape(P, HCH, TT, TW).transpose(0, 2, 1, 3)).astype(BF)

    pos = np.asarray(positions).reshape(-1).astype(np.float64)
    inv = ROPE_BASE ** (-np.arange(0, DR, 2, dtype=np.float64) / DR)
    fr = pos[:, None] * inv[None, :]                      # [T, 32]
    c32 = np.cos(fr).T.astype(np.float32)                 # [32, T]
    s32 = np.sin(fr).T.astype(np.float32)
    cosf = np.concatenate([c32] * 4, 0)
    sinf = np.concatenate([-s32, s32, -s32, s32], 0)

    r = np.arange(P)[:, None]
    c = np.arange(TW)[None, :]
    masks = np.stack([np.where(c >= r + j * P, 0.0, NEG) for j in range(4)],
                     1).astype(np.float32)                # [128, 4, 512]

    def tilemaj(a, chunks):
        # [rows, cols] -> [128, chunks, cols], partition-major
        return np.ascontiguousarray(
            a.reshape(chunks, P, -1).transpose(1, 0, 2)).astype(BF)

    w_in_ln = np.asarray(w_in_ln, np.float32)
    wqf = (np.asarray(w_q, np.float32) * w_in_ln[:, None] * SCALING
           ).reshape(HID, H, DQK)
    wkvaf = np.asarray(w_kv_a, np.float32) * w_in_ln[:, None]
    kpe_w = wkvaf[:, KVR:]
    pe_pair = np.concatenate([kpe_w[:, 0::2], kpe_w[:, 1::2]], 1)
    wkva_p = np.concatenate([wkvaf[:, :KVR], pe_pair, pe_pair], 1)
    wkvbf = (np.asarray(w_kv_b, np.float32)
             * np.asarray(w_kv_a_ln, np.float32)[:, None]).reshape(KVR, H, DN + DV)
    w_post_ln = np.asarray(w_post_ln, np.float32)
    wgf = np.asarray(w_gate, np.float32) * w_post_ln[:, None]
    wuf = np.asarray(w_up, np.float32) * w_post_ln[:, None]
    wdf = np.asarray(w_down, np.float32)
    wof = np.asarray(w_o, np.float32).reshape(H, DV, HID)

    in_maps = []
    for core in range(NC_N):
        hs = [2 * core, 2 * core + 1]
        nopes = np.concatenate([wqf[:, h, :DN] for h in hs], 1)
        pes = []
        for h in hs:
            pe = wqf[:, h, DN:]
            pes += [pe[:, 0::2], pe[:, 1::2]]
        wq_c = np.concatenate([nopes] + pes, 1)
        wkvb_c = np.concatenate(
            [wkvbf[:, hs[0], :DN], wkvbf[:, hs[1], :DN],
             wkvbf[:, hs[0], DN:], wkvbf[:, hs[1], DN:]], 1)   # [512, 512]
        in_maps.append({
            "htb": htb,
            "hto": np.ascontiguousarray(
                hTt[:, :, core * TO:(core + 1) * TO]).astype(BF),
            "wq": tilemaj(wq_c, HCH),
            "wkva": tilemaj(wkva_p, HCH),
            "wkvb": tilemaj(wkvb_c, KC),
            "wo": np.ascontiguousarray(
                np.concatenate([wof[h] for h in hs], 0).reshape(
                    HPC, P, HID).transpose(1, 0, 2)).astype(BF),
            "wg": tilemaj(wgf[:, core * FPC:(core + 1) * FPC], HCH),
            "wu": tilemaj(wuf[:, core * FPC:(core + 1) * FPC], HCH),
            "wd": tilemaj(wdf[core * FPC:(core + 1) * FPC, :], FPC // P),
            "cosf": cosf.astype(BF),
            "sinf": sinf.astype(BF),
            "masks": masks,
        })
    return in_maps


def kernel(**inputs):
    if "nc" not in _CACHE:
        _CACHE["nc"] = _build()
    nc = _CACHE["nc"]
    in_maps = _prep(**inputs)
    res = run_bass_kernel_spmd(nc, in_maps, core_ids=list(range(NC_N)))
    xT = np.empty((HCH, NC_N, 16, T), np.float32)
    for c in range(NC_N):
        slab = np.asarray(res.results[c]["o"], dtype=np.float32)
        slab = slab.reshape(16, TT, HCH, TW)          # [p, t, o, col]
        xT[:, c, :, :] = slab.transpose(2, 0, 1, 3).reshape(HCH, 16, T)
    return np.ascontiguousarray(
        xT.reshape(HID, T).T).reshape(B, S, HID)


# revision 18
# speedup vs baseline: 1.2993x; 1.0258x over previous
"""DeepseekV2 decoder layer on 8 TRN2 NeuronCores (Bass/Tile).

Sharding: TP over heads (2/core) for q/kv_b/attention/o_proj, kv_a sharded
over tokens (256/core) + AllGather, TP over INTER (1024/core) for the MLP.
Chunked AllReduce after o_proj and chunked ReduceScatter after down_proj,
overlapped with compute.

Internal layout is feature-major ("transposed"): activations live as
[feature, token] so every matmul output feeds the next as `rhs` without any
on-device transpose. RoPE pair-swaps, RMSNorm weight folding, the softmax
scaling, and cos/sin tables are all folded into host-side weight prep.

All DRAM tensors are pre-tiled on the host to [128, ...] partition-major
layout so every load/store is a single large dma_start (128 fat
descriptors) instead of hundreds of small ones.
"""

import numpy as np
import ml_dtypes

import concourse.bass as bass
import concourse.mybir as mybir
import concourse.tile as tile
from concourse import bacc
from concourse.bass_utils import run_bass_kernel_spmd

BF = ml_dtypes.bfloat16

B, S, HID = 2, 1024, 2048
T = B * S                      # 2048 tokens
H = 16
DN, DR = 128, 64
DQK = DN + DR
DV = 128
KVR = 512
INTER = 8192
EPS = 1e-6
ROPE_BASE = 10000.0
SCALING = DQK ** -0.5

NC_N = 8
HPC = H // NC_N                # 2 heads per core
FPC = INTER // NC_N            # 1024 inter per core
P = 128
HCH = HID // P                 # 16 hid chunks
TT = 4                         # token chunks of 512
TW = T // TT                   # 512
TO = T // NC_N                 # 256 own tokens for kv_a
KT = S // P                    # 8 k-tiles of 128 per batch
QT = S // TW                   # 2 q-chunks of 512 per batch
KC = KVR // P                  # 4 kv-lora chunks
NEG = -30000.0

f32 = mybir.dt.float32
bf16 = mybir.dt.bfloat16
ADD = mybir.AluOpType.add
MUL = mybir.AluOpType.mult
BYP = mybir.AluOpType.bypass
AF = mybir.ActivationFunctionType

_CACHE = {}


def _build():
    nc = bacc.Bacc("TRN2", target_bir_lowering=False, debug=False, num_devices=NC_N)
    dp = lambda n, sh, dt: nc.dram_tensor(n, sh, dt, kind="ExternalInput")
    htb = dp("htb", [P, TT, HCH, TW], bf16)     # hidden^T, chunk-tiled
    hto = dp("hto", [P, HCH, TO], bf16)         # own-token slice of hidden^T
    wq = dp("wq", [P, HCH, HPC * DQK], bf16)    # [h0n,h1n,h0x1,h0x2,h1x1,h1x2]
    wkva = dp("wkva", [P, HCH, KVR + 2 * DR], bf16)  # kv cols + pe dup'd twice
    wkvb = dp("wkvb", [P, KC, HPC * (DN + DV)], bf16)
    wo = dp("wo", [P, HPC, HID], bf16)
    wg = dp("wg", [P, HCH, FPC], bf16)
    wu = dp("wu", [P, HCH, FPC], bf16)
    wd = dp("wd", [P, FPC // P, HID], bf16)
    cosf = dp("cosf", [P, T], bf16)
    sinf = dp("sinf", [P, T], bf16)
    masks = dp("masks", [P, 4, TW], f32)
    out = nc.dram_tensor("o", [16, TT * HCH * TW], bf16, kind="ExternalOutput")
    rg = [list(range(NC_N))]

    with tile.TileContext(nc) as tc:
        with tc.tile_pool(name="const", bufs=1) as cpool, \
             tc.tile_pool(name="dram", bufs=1, space="DRAM") as dram, \
             tc.tile_pool(name="mlpw", bufs=1) as mlpw:
            ones_col = cpool.tile([P, 1], bf16)
            nc.vector.memset(ones_col[:], 1.0)
            ones_row = cpool.tile([1, P], bf16)
            nc.vector.memset(ones_row[:], 1.0)
            epsb = cpool.tile([1, 1], f32)
            nc.vector.memset(epsb[:], EPS)

            ag_in = dram.tile([P, KC * TO], bf16, name="ag_in")
            ag_out = dram.tile([NC_N * P, KC * TO], bf16, addr_space="Shared",
                               name="ag_out")
            ar_in = [dram.tile([P, 2, HCH, TW], bf16, name=f"ar_in{a}")
                     for a in range(2)]
            ar_out = [dram.tile([P, 2, HCH, TW], bf16, addr_space="Shared",
                                name=f"ar_out{a}") for a in range(2)]
            # chunks 0-2 reduce-scatter 2MB; chunk 3 in two 1MB halves (tail)
            rs_in = [dram.tile([P, HCH, TW], bf16, name=f"rs_in{t}")
                     for t in range(TT - 1)]
            rs_in += [dram.tile([P, HCH // 2, TW], bf16, name=f"rs_in3{i}")
                      for i in range(2)]
            rs_out = [dram.tile([16, HCH * TW], bf16, name=f"rs_out{t}")
                      for t in range(TT - 1)]
            rs_out += [dram.tile([16, HCH // 2 * TW], bf16, name=f"rs_out3{i}")
                       for i in range(2)]

            # ============ Phase A: projections + attention ============
            with tc.tile_pool(name="akeep", bufs=1) as akeep, \
                 tc.tile_pool(name="awrk", bufs=2) as awrk, \
                 tc.tile_pool(name="arow", bufs=2) as arow, \
                 tc.tile_pool(name="aps", bufs=1, space="PSUM") as aps:

                # survives A1 -> A2
                qsb = akeep.tile([P, 3, T], bf16)          # 12K
                kpe2 = akeep.tile([P, T], bf16)            # 4K (dup'd rope rows)

                # ---- A1: input norm + q/kv_a projections ----
                with tc.tile_pool(name="a1", bufs=1) as a1:
                    hto_sb = a1.tile([P, HCH, TO], bf16)
                    nc.scalar.dma_start(hto_sb[:], hto.ap())
                    wkva_sb = a1.tile([P, HCH, KVR + 2 * DR], bf16)
                    nc.scalar.dma_start(wkva_sb[:], wkva.ap())
                    wq_sb = a1.tile([P, HCH, HPC * DQK], bf16)
                    nc.scalar.dma_start(wq_sb[:], wq.ap())

                    # -- kv_a for OWN 256 tokens (sharded), then AllGather --
                    lat_own = a1.tile([P, KC, TO], bf16)
                    ss2p = aps.tile([1, TO], f32, tag="ss2", bufs=1, name="ss2p")
                    sqg2 = awrk.tile([P, TO], bf16, tag="sq", name="sqg2")
                    for f in range(KC):
                        lp = aps.tile([P, TO], f32, tag="big", bufs=2, name="lp")
                        for o in range(HCH):
                            nc.tensor.matmul(lp[:], wkva_sb[:, o, f * P:(f + 1) * P],
                                             hto_sb[:, o, :],
                                             start=(o == 0), stop=(o == HCH - 1))
                        nc.vector.tensor_copy(out=lat_own[:, f, :], in_=lp[:])
                        if f == 0:
                            nc.vector.tensor_tensor(sqg2[:], lat_own[:, f, :],
                                                    lat_own[:, f, :], MUL)
                        else:
                            sq2 = awrk.tile([P, TO], bf16, tag="sq", name="sq2")
                            nc.vector.tensor_tensor(sq2[:], lat_own[:, f, :],
                                                    lat_own[:, f, :], MUL)
                            nc.vector.tensor_tensor(sqg2[:], sqg2[:], sq2[:], ADD)
                    nc.tensor.matmul(ss2p[:], ones_col[:], sqg2[:],
                                     start=True, stop=True)
                    srow2 = arow.tile([1, TO], f32, tag="srow", name="srow2")
                    nc.scalar.activation(srow2[:], ss2p[:], AF.Sqrt,
                                         bias=epsb[:], scale=1.0 / KVR)
                    rrow2 = arow.tile([1, TO], f32, tag="rrow", name="rrow2")
                    nc.vector.reciprocal(rrow2[:], srow2[:])
                    rb2 = arow.tile([1, TO], bf16, tag="rb", name="rb2")
                    nc.vector.tensor_copy(out=rb2[:], in_=rrow2[:])
                    bcp2 = aps.tile([P, TO], f32, tag="vp", bufs=1, name="bcp2")
                    nc.tensor.matmul(bcp2[:], ones_row[:], rb2[:],
                                     start=True, stop=True)
                    bc2 = a1.tile([P, TO], f32, name="bc2")
                    nc.vector.tensor_copy(out=bc2[:], in_=bcp2[:])
                    agblk = a1.tile([P, KC, TO], bf16, name="agblk")
                    for f in range(KC):
                        nc.vector.tensor_tensor(agblk[:, f, :], lat_own[:, f, :],
                                                bc2[:], MUL)
                    nc.sync.dma_start(ag_in[:], agblk[:])
                    nc.gpsimd.collective_compute(
                        "AllGather", BYP, ins=[ag_in[:].opt()],
                        outs=[ag_out[:].opt()], replica_groups=rg)

                    # -- input rmsnorm scale + q proj + k_pe, per token chunk --
                    for t in range(TT):
                        ht_t = a1.tile([P, HCH, TW], bf16, tag="ht", bufs=2,
                                       name="ht_t")
                        nc.sync.dma_start(ht_t[:], htb.ap()[:, t, :, :])
                        # sum of squares: quad-group on vector, 4 matmul reduces
                        ssp = aps.tile([1, TW], f32, tag="ss", bufs=1, name="ssp")
                        for g in range(4):
                            sqg = awrk.tile([P, TW], bf16, tag="sqg", name="sqg")
                            for k in range(4):
                                o = 4 * g + k
                                if k == 0:
                                    nc.vector.tensor_tensor(sqg[:], ht_t[:, o, :],
                                                            ht_t[:, o, :], MUL)
                                else:
                                    sq = awrk.tile([P, TW], bf16, tag="sq",
                                                   name="sq")
                                    nc.vector.tensor_tensor(sq[:], ht_t[:, o, :],
                                                            ht_t[:, o, :], MUL)
                                    nc.vector.tensor_tensor(sqg[:], sqg[:], sq[:],
                                                            ADD)
                            nc.tensor.matmul(ssp[:], ones_col[:], sqg[:],
                                             start=(g == 0), stop=(g == 3))
                        srow = arow.tile([1, TW], f32, tag="srow", name="srow")
                        nc.scalar.activation(srow[:], ssp[:], AF.Sqrt,
                                             bias=epsb[:], scale=1.0 / HID)
                        rrow = arow.tile([1, TW], f32, tag="rrow", name="rrow")
                        nc.vector.reciprocal(rrow[:], srow[:])
                        rb = arow.tile([1, TW], bf16, tag="rb", name="rb")
                        nc.vector.tensor_copy(out=rb[:], in_=rrow[:])
                        bcp = aps.tile([P, TW], f32, tag="big", bufs=2, name="bcp")
                        nc.tensor.matmul(bcp[:], ones_row[:], rb[:],
                                         start=True, stop=True)
                        bc1 = a1.tile([P, TW], f32, tag="bc1", bufs=2, name="bc1")
                        nc.vector.tensor_copy(out=bc1[:], in_=bcp[:])

                        # q projection (scaled by r1; SCALING folded into wq)
                        for f in range(3):
                            qp = aps.tile([P, TW], f32, tag="big", bufs=2, name="qp")
                            for o in range(HCH):
                                nc.tensor.matmul(qp[:], wq_sb[:, o, f * P:(f + 1) * P],
                                                 ht_t[:, o, :],
                                                 start=(o == 0), stop=(o == HCH - 1))
                            nc.vector.tensor_tensor(qsb[:, f, t * TW:(t + 1) * TW],
                                                    qp[:], bc1[:], MUL)
                        # k_pe (duplicated rows for both attention heads) * r1
                        kp2 = aps.tile([P, TW], f32, tag="big", bufs=2, name="kp2")
                        for o in range(HCH):
                            nc.tensor.matmul(kp2[:], wkva_sb[:, o, KVR:KVR + 2 * DR],
                                             ht_t[:, o, :],
                                             start=(o == 0), stop=(o == HCH - 1))
                        nc.vector.tensor_tensor(kpe2[:, t * TW:(t + 1) * TW],
                                                kp2[:], bc1[:], MUL)

                # ---- A2: rope, kv_b, attention, o_proj (+AR) ----
                with tc.tile_pool(name="a2", bufs=1) as a2:
                    kva2 = a2.tile([P, NC_N, KC, TO], bf16)      # 16K
                    for r in range(NC_N):
                        nc.sync.dma_start(kva2[:, r, :, :],
                                          ag_out[r * P:(r + 1) * P, :])
                    wkvb_sb = a2.tile([P, KC, HPC * (DN + DV)], bf16)
                    nc.sync.dma_start(wkvb_sb[:], wkvb.ap())
                    wo_sb = a2.tile([P, HPC, HID], bf16)
                    nc.scalar.dma_start(wo_sb[:], wo.ap())
                    cs = a2.tile([P, T], bf16)
                    nc.scalar.dma_start(cs[:], cosf.ap())
                    sn = a2.tile([P, T], bf16)
                    nc.scalar.dma_start(sn[:], sinf.ap())
                    msk = a2.tile([P, 4, TW], f32)
                    nc.scalar.dma_start(msk[:], masks.ap())
                    # prefetch the big MLP weights early (consumed in phase B)
                    wg_sb = mlpw.tile([P, HCH, FPC], bf16)       # 32K
                    nc.scalar.dma_start(wg_sb[:], wg.ap())
                    wu_sb = mlpw.tile([P, HCH, FPC], bf16)       # 32K
                    nc.scalar.dma_start(wu_sb[:], wu.ap())

                    # kv_b: k_nope (transposed out) + v (natural out)
                    knope = a2.tile([P, HPC, T], bf16)
                    for h in range(HPC):
                        for t2 in range(NC_N):
                            kp = aps.tile([P, TO], f32, tag="big", bufs=2, name="kp")
                            for c in range(KC):
                                nc.tensor.matmul(kp[:],
                                                 wkvb_sb[:, c, h * P:(h + 1) * P],
                                                 kva2[:, t2, c, :],
                                                 start=(c == 0), stop=(c == KC - 1))
                            nc.vector.tensor_copy(
                                out=knope[:, h, t2 * TO:(t2 + 1) * TO], in_=kp[:])
                    vnat = a2.tile([P, T // P, HPC * DV], bf16)
                    for to in range(T // P):
                        vp = aps.tile([P, HPC * DV], f32, tag="vp", bufs=1, name="vp")
                        for c in range(KC):
                            nc.tensor.matmul(vp[:],
                                             kva2[:, to // 2, c,
                                                  (to % 2) * P:(to % 2 + 1) * P],
                                             wkvb_sb[:, c, HPC * DN:],
                                             start=(c == 0), stop=(c == KC - 1))
                        nc.vector.tensor_copy(out=vnat[:, to, :], in_=vp[:])

                    # rope in place: qsb[:,2,:] rows are [h0x1,h0x2,h1x1,h1x2],
                    # kpe2 rows are [x1,x2,x1,x2]; cs=[c,c,c,c], sn=[-s,s,-s,s]
                    for src in (qsb[:, 2, :], kpe2[:]):
                        swp = a2.tile([P, T], bf16, tag="swp", bufs=2, name="swp")
                        for g in range(4):
                            half = 32 if g % 2 == 0 else -32
                            nc.sync.dma_start(swp[g * 32:(g + 1) * 32, :],
                                              src[g * 32 + half:(g + 1) * 32 + half, :])
                        rtmp = a2.tile([P, T], bf16, tag="rtmp", bufs=2, name="rtmp")
                        nc.vector.tensor_tensor(rtmp[:], src, cs[:], MUL)
                        nc.vector.tensor_tensor(src, swp[:], sn[:], MUL)
                        nc.vector.tensor_tensor(src, src, rtmp[:], ADD)

                    # attention (scores transposed: [k, q]) + o_proj partial + AR
                    for b in range(B):
                        for qt in range(QT):
                            tt = b * QT + qt
                            qc0 = b * S + qt * TW
                            nkt = 4 * qt + 4
                            attn_t = a2.tile([P, HPC, TW], bf16, tag="attn",
                                             bufs=2, name="attn_t")
                            for h in range(HPC):
                                dnp = aps.tile([1, TW], f32, tag="den", bufs=1,
                                               name="dnp")
                                atp = aps.tile([P, TW], f32, tag="att", bufs=2,
                                               name="atp")
                                exs = [None] * nkt

                                def consume(kt):
                                    nc.tensor.matmul(dnp[:], ones_col[:], exs[kt][:],
                                                     start=(kt == 0),
                                                     stop=(kt == nkt - 1))
                                    nc.tensor.matmul(atp[:],
                                                     vnat[:, b * KT + kt,
                                                          h * DV:(h + 1) * DV],
                                                     exs[kt][:],
                                                     start=(kt == 0),
                                                     stop=(kt == nkt - 1))

                                for kt in range(nkt):
                                    kc0 = b * S + kt * P
                                    scp = aps.tile([P, TW], f32, tag="big", bufs=2,
                                                   name="scp")
                                    nc.tensor.matmul(scp[:],
                                                     knope[:, h, kc0:kc0 + P],
                                                     qsb[:, h, qc0:qc0 + TW],
                                                     start=True, stop=False)
                                    nc.tensor.matmul(
                                        scp[:],
                                        kpe2[h * DR:(h + 1) * DR, kc0:kc0 + P],
                                        qsb[h * DR:(h + 1) * DR, 2, qc0:qc0 + TW],
                                        start=False, stop=True)
                                    ex = awrk.tile([P, TW], bf16, tag="ex", bufs=4,
                                                   name="ex")
                                    j = kt - 4 * qt
                                    if j >= 0:
                                        mtmp = awrk.tile([P, TW], f32, tag="mt",
                                                         name="mtmp")
                                        nc.vector.tensor_tensor(mtmp[:], scp[:],
                                                                msk[:, j, :], ADD)
                                        nc.scalar.activation(ex[:], mtmp[:], AF.Exp)
                                    else:
                                        nc.scalar.activation(ex[:], scp[:], AF.Exp)
                                    exs[kt] = ex
                                    if kt >= 2:
                                        consume(kt - 2)
                                consume(max(nkt - 2, 0))
                                if nkt > 1:
                                    consume(nkt - 1)
                                drow = arow.tile([1, TW], bf16, tag="rb", name="drow")
                                with nc.allow_low_precision(reason="softmax denom"):
                                    nc.vector.reciprocal(drow[:], dnp[:])
                                dbp = aps.tile([P, TW], f32, tag="big", bufs=2,
                                               name="dbp")
                                nc.tensor.matmul(dbp[:], ones_row[:], drow[:],
                                                 start=True, stop=True)
                                dbc = awrk.tile([P, TW], f32, tag="mt", name="dbc")
                                nc.vector.tensor_copy(out=dbc[:], in_=dbp[:])
                                nc.vector.tensor_tensor(
                                    attn_t[:, h, :], atp[:], dbc[:], MUL)
                            # o_proj partial for this token chunk
                            oall = a2.tile([P, HCH, TW], bf16, tag="oall", bufs=1,
                                           name="oall")
                            for ho in range(HCH):
                                op = aps.tile([P, TW], f32, tag="big", bufs=2,
                                              name="op")
                                for h in range(HPC):
                                    nc.tensor.matmul(op[:],
                                                     wo_sb[:, h, ho * P:(ho + 1) * P],
                                                     attn_t[:, h, :],
                                                     start=(h == 0),
                                                     stop=(h == HPC - 1))
                                nc.vector.tensor_copy(out=oall[:, ho, :], in_=op[:])
                            nc.sync.dma_start(ar_in[tt // 2][:, tt % 2, :, :],
                                              oall[:])
                            if tt % 2 == 1:
                                nc.gpsimd.collective_compute(
                                    "AllReduce", ADD, ins=[ar_in[tt // 2][:].opt()],
                                    outs=[ar_out[tt // 2][:].opt()],
                                    replica_groups=rg)

            # ============ Phase B: residual + norm + MLP ============
            with tc.tile_pool(name="bbig", bufs=1) as bbig, \
                 tc.tile_pool(name="bwrk", bufs=2) as bwrk, \
                 tc.tile_pool(name="brow", bufs=1) as brow, \
                 tc.tile_pool(name="bps", bufs=1, space="PSUM") as bps:

                wd_sb = bbig.tile([P, FPC // P, HID], bf16)  # 32K
                nc.scalar.dma_start(wd_sb[:], wd.ap())

                for t in range(TT):
                    # x = hidden + attn_out; later x/8 in place
                    x = bbig.tile([P, HCH, TW], bf16, name="x", tag="x", bufs=2)
                    nc.sync.dma_start(x[:], htb.ap()[:, t, :, :])
                    arall = bbig.tile([P, HCH, TW], bf16, name="arall", tag="ar",
                                      bufs=1)
                    nc.sync.dma_start(arall[:], ar_out[t // 2][:, t % 2, :, :])
                    ssp3 = bps.tile([1, TW], f32, tag="ss", bufs=1, name="ssp3")
                    for g in range(4):
                        sqg3 = bwrk.tile([P, TW], bf16, tag="sqg3", bufs=2,
                                         name="sqg3")
                        for k in range(4):
                            o = 4 * g + k
                            nc.vector.tensor_tensor(x[:, o, :], x[:, o, :],
                                                    arall[:, o, :], ADD)
                            if k == 0:
                                nc.vector.tensor_tensor(sqg3[:], x[:, o, :],
                                                        x[:, o, :], MUL)
                            else:
                                sq3 = bwrk.tile([P, TW], bf16, tag="sq3", bufs=2,
                                                name="sq3")
                                nc.vector.tensor_tensor(sq3[:], x[:, o, :],
                                                        x[:, o, :], MUL)
                                nc.vector.tensor_tensor(sqg3[:], sqg3[:], sq3[:],
                                                        ADD)
                        nc.tensor.matmul(ssp3[:], ones_col[:], sqg3[:],
                                         start=(g == 0), stop=(g == 3))
                    srow3 = brow.tile([1, TW], f32, tag="srow3", name="srow3")
                    nc.scalar.activation(srow3[:], ssp3[:], AF.Sqrt,
                                         bias=epsb[:], scale=1.0 / HID)
                    rrow3 = brow.tile([1, TW], f32, tag="rrow3", name="rrow3")
                    nc.vector.reciprocal(rrow3[:], srow3[:])
                    rb3 = brow.tile([1, TW], bf16, tag="rb3", name="rb3")
                    nc.vector.tensor_copy(out=rb3[:], in_=rrow3[:])
                    bcp3 = bps.tile([P, TW], f32, tag="gu", bufs=4, name="bcp3")
                    nc.tensor.matmul(bcp3[:], ones_row[:], rb3[:], start=True,
                                     stop=True)
                    bc3 = bwrk.tile([P, TW], f32, tag="bc3", bufs=1, name="bc3")
                    nc.vector.tensor_copy(out=bc3[:], in_=bcp3[:])
                    h2 = bbig.tile([P, HCH, TW], bf16, name="h2", tag="h2", bufs=1)
                    for o in range(HCH):
                        nc.vector.tensor_tensor(h2[:, o, :], x[:, o, :], bc3[:], MUL)
                    # x -> x/8 in place (folded residual for ReduceScatter)
                    nc.vector.tensor_scalar_mul(x[:, :, :], x[:, :, :], 0.125)

                    # gate/up/silu
                    act = bbig.tile([P, FPC // P, TW], bf16, name="act", tag="act",
                                    bufs=1)
                    for fi in range(FPC // P):
                        gp = bps.tile([P, TW], f32, tag="gu", bufs=4, name="gp")
                        for o in range(HCH):
                            nc.tensor.matmul(gp[:], wg_sb[:, o, fi * P:(fi + 1) * P],
                                             h2[:, o, :],
                                             start=(o == 0), stop=(o == HCH - 1))
                        up = bps.tile([P, TW], f32, tag="gu", bufs=4, name="up")
                        for o in range(HCH):
                            nc.tensor.matmul(up[:], wu_sb[:, o, fi * P:(fi + 1) * P],
                                             h2[:, o, :],
                                             start=(o == 0), stop=(o == HCH - 1))
                        gs = bwrk.tile([P, TW], f32, tag="gs", bufs=2, name="gs")
                        nc.scalar.activation(gs[:], gp[:], AF.Silu)
                        nc.vector.tensor_tensor(act[:, fi, :], up[:], gs[:], MUL)

                    # down projection partial (+x/8) + RS
                    # last chunk reduces in two 1MB halves to shrink the tail
                    dall = bbig.tile([P, HCH, TW], bf16, name="dall", tag="dall",
                                     bufs=1)
                    for ho in range(HCH):
                        dpp = bps.tile([P, TW], f32, tag="d", bufs=2, name="dpp")
                        for c in range(FPC // P):
                            nc.tensor.matmul(dpp[:], wd_sb[:, c, ho * P:(ho + 1) * P],
                                             act[:, c, :],
                                             start=(c == 0), stop=(c == FPC // P - 1))
                        nc.vector.tensor_tensor(dall[:, ho, :], dpp[:], x[:, ho, :],
                                                ADD)
                        if t == TT - 1 and ho == HCH // 2 - 1:
                            nc.sync.dma_start(rs_in[3][:], dall[:, :HCH // 2, :])
                            nc.gpsimd.collective_compute(
                                "ReduceScatter", ADD, ins=[rs_in[3][:].opt()],
                                outs=[rs_out[3][:].opt()], replica_groups=rg)
                            nc.sync.dma_start(
                                out.ap()[:, 3 * HCH * TW:
                                         3 * HCH * TW + HCH // 2 * TW],
                                rs_out[3][:])
                    if t < TT - 1:
                        nc.sync.dma_start(rs_in[t][:], dall[:])
                        nc.gpsimd.collective_compute(
                            "ReduceScatter", ADD, ins=[rs_in[t][:].opt()],
                            outs=[rs_out[t][:].opt()], replica_groups=rg)
                        nc.sync.dma_start(
                            out.ap()[:, t * HCH * TW:(t + 1) * HCH * TW],
                            rs_out[t][:])
                    else:
                        nc.sync.dma_start(rs_in[4][:], dall[:, HCH // 2:, :])
                        nc.gpsimd.collective_compute(
                            "ReduceScatter", ADD, ins=[rs_in[4][:].opt()],
                            outs=[rs_out[4][:].opt()], replica_groups=rg)
                        nc.sync.dma_start(
                            out.ap()[:, 3 * HCH * TW + HCH // 2 * TW:],
                            rs_out[4][:])
    nc.compile()
    return nc


def _prep(hidden_states, positions, w_in_ln, w_q, w_kv_a, w_kv_a_ln,
          w_kv_b, w_o, w_post_ln, w_gate, w_up, w_down):
    hT = np.ascontiguousarray(
        np.asarray(hidden_states, np.float32).reshape(T, HID).T)
    hTt = hT.reshape(HCH, P, T).transpose(1, 0, 2)          # [128, 16, T]
    htb = np.ascontiguousarray(
        hTt.reshape(P, HCH, TT, TW).transpose(0, 2, 1, 3)).astype(BF)

    pos = np.asarray(positions).reshape(-1).astype(np.float64)
    inv = ROPE_BASE ** (-np.arange(0, DR, 2, dtype=np.float64) / DR)
    fr = pos[:, None] * inv[None, :]                      # [T, 32]
    c32 = np.cos(fr).T.astype(np.float32)                 # [32, T]
    s32 = np.sin(fr).T.astype(np.float32)
    cosf = np.concatenate([c32] * 4, 0)
    sinf = np.concatenate([-s32, s32, -s32, s32], 0)

    r = np.arange(P)[:, None]
    c = np.arange(TW)[None, :]
    masks = np.stack([np.where(c >= r + j * P, 0.0, NEG) for j in range(4)],
                     1).astype(np.float32)                # [128, 4, 512]

    def tilemaj(a, chunks):
        # [rows, cols] -> [128, chunks, cols], partition-major
        return np.ascontiguousarray(
            a.reshape(chunks, P, -1).transpose(1, 0, 2)).astype(BF)

    w_in_ln = np.asarray(w_in_ln, np.float32)
    wqf = (np.asarray(w_q, np.float32) * w_in_ln[:, None] * SCALING
           ).reshape(HID, H, DQK)
    wkvaf = np.asarray(w_kv_a, np.float32) * w_in_ln[:, None]
    kpe_w = wkvaf[:, KVR:]
    pe_pair = np.concatenate([kpe_w[:, 0::2], kpe_w[:, 1::2]], 1)
    wkva_p = np.concatenate([wkvaf[:, :KVR], pe_pair, pe_pair], 1)
    wkvbf = (np.asarray(w_kv_b, np.float32)
             * np.asarray(w_kv_a_ln, np.float32)[:, None]).reshape(KVR, H, DN + DV)
    w_post_ln = np.asarray(w_post_ln, np.float32)
    wgf = np.asarray(w_gate, np.float32) * w_post_ln[:, None]
    wuf = np.asarray(w_up, np.float32) * w_post_ln[:, None]
    wdf = np.asarray(w_down, np.float32)
    wof = np.asarray(w_o, np.float32).reshape(H, DV, HID)

    in_maps = []
    for core in range(NC_N):
        hs = [2 * core, 2 * core + 1]
        nopes = np.concatenate([wqf[:, h, :DN] for h in hs], 1)
        pes = []
        for h in hs:
            pe = wqf[:, h, DN:]
            pes += [pe[:, 0::2], pe[:, 1::2]]
        wq_c = np.concatenate([nopes] + pes, 1)
        wkvb_c = np.concatenate(
            [wkvbf[:, hs[0], :DN], wkvbf[:, hs[1], :DN],
             wkvbf[:, hs[0], DN:], wkvbf[:, hs[1], DN:]], 1)   # [512, 512]
        in_maps.append({
            "htb": htb,
            "hto": np.ascontiguousarray(
                hTt[:, :, core * TO:(core + 1) * TO]).astype(BF),
            "wq": tilemaj(wq_c, HCH),
            "wkva": tilemaj(wkva_p, HCH),
            "wkvb": tilemaj(wkvb_c, KC),
            "wo": np.ascontiguousarray(
                np.concatenate([wof[h] for h in hs], 0).reshape(
                    HPC, P, HID).transpose(1, 0, 2)).astype(BF),
            "wg": tilemaj(wgf[:, core * FPC:(core + 1) * FPC], HCH),
            "wu": tilemaj(wuf[:, core * FPC:(core + 1) * FPC], HCH),
            "wd": tilemaj(wdf[core * FPC:(core + 1) * FPC, :], FPC // P),
            "cosf": cosf.astype(BF),
            "sinf": sinf.astype(BF),
            "masks": masks,
        })
    return in_maps


def kernel(**inputs):
    if "nc" not in _CACHE:
        _CACHE["nc"] = _build()
    nc = _CACHE["nc"]
    in_maps = _prep(**inputs)
    res = run_bass_kernel_spmd(nc, in_maps, core_ids=list(range(NC_N)))
    xT = np.empty((HCH, NC_N, 16, T), np.float32)
    for c in range(NC_N):
        slab = np.asarray(res.results[c]["o"], dtype=np.float32)
        slab = slab.reshape(16, TT, HCH, TW)          # [p, t, o, col]
        xT[:, c, :, :] = slab.transpose(2, 0, 1, 3).reshape(HCH, 16, T)
    return np.ascontiguousarray(
        xT.reshape(HID, T).T).reshape(B, S, HID)


# revision 32
# speedup vs baseline: 1.3027x; 1.0026x over previous
"""DeepseekV2 decoder layer on 8 TRN2 NeuronCores (Bass/Tile).

Sharding: TP over heads (2/core) for q/kv_b/attention/o_proj, kv_a sharded
over tokens (256/core) + AllGather, TP over INTER (1024/core) for the MLP.
Chunked AllReduce after o_proj and chunked ReduceScatter after down_proj,
overlapped with compute.

Internal layout is feature-major ("transposed"): activations live as
[feature, token] so every matmul output feeds the next as `rhs` without any
on-device transpose. RoPE pair-swaps, RMSNorm weight folding, the softmax
scaling, and cos/sin tables are all folded into host-side weight prep.

All DRAM tensors are pre-tiled on the host to [128, ...] partition-major
layout so every load/store is a single large dma_start (128 fat
descriptors) instead of hundreds of small ones.
"""

import numpy as np
import ml_dtypes

import concourse.bass as bass
import concourse.mybir as mybir
import concourse.tile as tile
from concourse import bacc
from concourse.bass_utils import run_bass_kernel_spmd

BF = ml_dtypes.bfloat16

B, S, HID = 2, 1024, 2048
T = B * S                      # 2048 tokens
H = 16
DN, DR = 128, 64
DQK = DN + DR
DV = 128
KVR = 512
INTER = 8192
EPS = 1e-6
ROPE_BASE = 10000.0
SCALING = DQK ** -0.5

NC_N = 8
HPC = H // NC_N                # 2 heads per core
FPC = INTER // NC_N            # 1024 inter per core
P = 128
HCH = HID // P                 # 16 hid chunks
TT = 4                         # token chunks of 512
TW = T // TT                   # 512
TO = T // NC_N                 # 256 own tokens for kv_a
KT = S // P                    # 8 k-tiles of 128 per batch
QT = S // TW                   # 2 q-chunks of 512 per batch
KC = KVR // P                  # 4 kv-lora chunks
NEG = -30000.0

f32 = mybir.dt.float32
bf16 = mybir.dt.bfloat16
ADD = mybir.AluOpType.add
MUL = mybir.AluOpType.mult
BYP = mybir.AluOpType.bypass
AF = mybir.ActivationFunctionType

_CACHE = {}


def _build():
    nc = bacc.Bacc("TRN2", target_bir_lowering=False, debug=False, num_devices=NC_N)
    dp = lambda n, sh, dt: nc.dram_tensor(n, sh, dt, kind="ExternalInput")
    htb = dp("htb", [P, TT, HCH, TW], bf16)     # hidden^T, chunk-tiled
    hto = dp("hto", [P, HCH, TO], bf16)         # own-token slice of hidden^T
    wq = dp("wq", [P, HCH, HPC * DQK], bf16)    # [h0n,h1n,h0x1,h0x2,h1x1,h1x2]
    wkva = dp("wkva", [P, HCH, KVR + DR], bf16)  # kv cols + pe (pair-split)
    wkvb = dp("wkvb", [P, KC, HPC * (DN + DV)], bf16)
    wo = dp("wo", [P, HPC, HID], bf16)
    wg = dp("wg", [P, HCH, FPC], bf16)
    wu = dp("wu", [P, HCH, FPC], bf16)
    wd = dp("wd", [P, FPC // P, HID], bf16)
    cosf = dp("cosf", [P, T], bf16)
    sinf = dp("sinf", [P, T], bf16)
    masks = dp("masks", [P, 4, TW], f32)
    out = nc.dram_tensor("o", [16, TT * HCH * TW], bf16, kind="ExternalOutput")
    rg = [list(range(NC_N))]

    with tile.TileContext(nc) as tc:
        with tc.tile_pool(name="const", bufs=1) as cpool, \
             tc.tile_pool(name="dram", bufs=1, space="DRAM") as dram, \
             tc.tile_pool(name="mlpw", bufs=1) as mlpw:
            ones_col = cpool.tile([P, 1], bf16)
            nc.vector.memset(ones_col[:], 1.0)
            ones_row = cpool.tile([1, P], bf16)
            nc.vector.memset(ones_row[:], 1.0)
            epsb = cpool.tile([1, 1], f32)
            nc.vector.memset(epsb[:], EPS)

            ag_in = dram.tile([P, KC * TO], bf16, name="ag_in")
            ag_out = dram.tile([NC_N * P, KC * TO], bf16, addr_space="Shared",
                               name="ag_out")
            # o_proj reduction as RS + AG (4x less wire than mesh AllReduce)
            ar_in = [dram.tile([P, HCH, TW], bf16, name=f"ar_in{t}")
                     for t in range(TT)]
            o_rs = [dram.tile([16, HCH * TW], bf16, name=f"o_rs{t}")
                    for t in range(TT)]
            o_ag = [dram.tile([P, HCH, TW], bf16, addr_space="Shared",
                              name=f"o_ag{t}") for t in range(TT)]
            # chunks 0-2 reduce-scatter 2MB; chunk 3 in two 1MB halves (tail)
            rs_in = [dram.tile([P, HCH, TW], bf16, name=f"rs_in{t}")
                     for t in range(TT - 1)]
            rs_in += [dram.tile([P, HCH // 2, TW], bf16, name=f"rs_in3{i}")
                      for i in range(2)]
            rs_out = [dram.tile([16, HCH * TW], bf16, name=f"rs_out{t}")
                      for t in range(TT - 1)]
            rs_out += [dram.tile([16, HCH // 2 * TW], bf16, name=f"rs_out3{i}")
                       for i in range(2)]

            # ============ Phase A: projections + attention ============
            with tc.tile_pool(name="akeep", bufs=1) as akeep, \
                 tc.tile_pool(name="awrk", bufs=2) as awrk, \
                 tc.tile_pool(name="arow", bufs=2) as arow, \
                 tc.tile_pool(name="aps", bufs=1, space="PSUM") as aps:

                # survives A1 -> A2
                qsb = akeep.tile([P, 3, T], bf16)          # 12K
                kpe2 = akeep.tile([P, T], bf16)            # 4K (dup'd rope rows)

                # ---- A1: input norm + q/kv_a projections ----
                with tc.tile_pool(name="a1", bufs=1) as a1:
                    hto_sb = a1.tile([P, HCH, TO], bf16)
                    nc.scalar.dma_start(hto_sb[:], hto.ap())
                    wkva_sb = a1.tile([P, HCH, KVR + DR], bf16)
                    nc.scalar.dma_start(wkva_sb[:], wkva.ap())
                    wq_sb = a1.tile([P, HCH, HPC * DQK], bf16)
                    nc.scalar.dma_start(wq_sb[:], wq.ap())

                    # -- kv_a for OWN 256 tokens (sharded), then AllGather --
                    lat_own = a1.tile([P, KC, TO], bf16)
                    ss2p = aps.tile([1, TW], f32, tag="ss", bufs=2, name="ss2p")
                    sqg2 = awrk.tile([P, TO], bf16, tag="sq", name="sqg2")
                    for f in range(KC):
                        lp = aps.tile([P, TO], f32, tag="big", bufs=2, name="lp")
                        for o in range(HCH):
                            nc.tensor.matmul(lp[:], wkva_sb[:, o, f * P:(f + 1) * P],
                                             hto_sb[:, o, :],
                                             start=(o == 0), stop=(o == HCH - 1))
                        nc.vector.tensor_copy(out=lat_own[:, f, :], in_=lp[:])
                        if f == 0:
                            nc.scalar.square(sqg2[:], lat_own[:, f, :])
                        else:
                            sq2 = awrk.tile([P, TO], bf16, tag="sq", name="sq2")
                            nc.scalar.square(sq2[:], lat_own[:, f, :])
                            nc.vector.tensor_tensor(sqg2[:], sqg2[:], sq2[:], ADD)
                    nc.tensor.matmul(ss2p[:, :TO], ones_col[:], sqg2[:],
                                     start=True, stop=True)
                    srow2 = arow.tile([1, TO], f32, tag="srow", name="srow2")
                    nc.scalar.activation(srow2[:], ss2p[:, :TO], AF.Sqrt,
                                         bias=epsb[:], scale=1.0 / KVR)
                    rrow2 = arow.tile([1, TO], f32, tag="rrow", name="rrow2")
                    nc.vector.reciprocal(rrow2[:], srow2[:])
                    rb2 = arow.tile([1, TO], bf16, tag="rb", name="rb2")
                    nc.vector.tensor_copy(out=rb2[:], in_=rrow2[:])
                    bcp2 = aps.tile([P, TW], f32, tag="att", bufs=2, name="bcp2")
                    nc.tensor.matmul(bcp2[:, :TO], ones_row[:], rb2[:],
                                     start=True, stop=True)
                    bc2 = a1.tile([P, TO], f32, name="bc2")
                    nc.vector.tensor_copy(out=bc2[:], in_=bcp2[:, :TO])
                    for f in range(KC):
                        nc.vector.tensor_tensor(lat_own[:, f, :], lat_own[:, f, :],
                                                bc2[:], MUL)
                    nc.sync.dma_start(ag_in[:], lat_own[:])
                    nc.gpsimd.collective_compute(
                        "AllGather", BYP, ins=[ag_in[:].opt()],
                        outs=[ag_out[:].opt()], replica_groups=rg)

                    # kv_b inputs live at akeep level: the kv_b matmuls can
                    # overlap the tail of A1 instead of waiting for pool swap
                    kva2 = akeep.tile([P, NC_N, KC, TO], bf16, name="kva2")
                    for r in range(NC_N):
                        nc.sync.dma_start(kva2[:, r, :, :],
                                          ag_out[r * P:(r + 1) * P, :])
                    wkvb_sb = akeep.tile([P, KC, HPC * (DN + DV)], bf16,
                                         name="wkvb_sb")
                    nc.sync.dma_start(wkvb_sb[:], wkvb.ap())
                    wo_sb = akeep.tile([P, HPC, HID], bf16, name="wo_sb")
                    nc.scalar.dma_start(wo_sb[:], wo.ap())

                    # -- input rmsnorm scale + q proj + k_pe, per token chunk --
                    for t in range(TT):
                        ht_t = a1.tile([P, HCH, TW], bf16, tag="ht", bufs=2,
                                       name="ht_t")
                        nc.sync.dma_start(ht_t[:], htb.ap()[:, t, :, :])
                        # sum of squares: scalar squares, quad-group adds on
                        # vector, 4 matmul reduces
                        ssp = aps.tile([1, TW], f32, tag="ss", bufs=2, name="ssp")
                        for g in range(4):
                            sqg = awrk.tile([P, TW], bf16, tag="sqg", name="sqg")
                            for k in range(4):
                                o = 4 * g + k
                                if k == 0:
                                    nc.scalar.square(sqg[:], ht_t[:, o, :])
                                else:
                                    sq = awrk.tile([P, TW], bf16, tag="sq",
                                                   name="sq")
                                    nc.scalar.square(sq[:], ht_t[:, o, :])
                                    nc.vector.tensor_tensor(sqg[:], sqg[:], sq[:],
                                                            ADD)
                            nc.tensor.matmul(ssp[:], ones_col[:], sqg[:],
                                             start=(g == 0), stop=(g == 3))
                        srow = arow.tile([1, TW], f32, tag="srow", name="srow")
                        nc.scalar.activation(srow[:], ssp[:], AF.Sqrt,
                                             bias=epsb[:], scale=1.0 / HID)
                        rrow = arow.tile([1, TW], f32, tag="rrow", name="rrow")
                        nc.vector.reciprocal(rrow[:], srow[:])
                        rb = arow.tile([1, TW], bf16, tag="rb", name="rb")
                        nc.vector.tensor_copy(out=rb[:], in_=rrow[:])
                        bcp = aps.tile([P, TW], f32, tag="big", bufs=2, name="bcp")
                        nc.tensor.matmul(bcp[:], ones_row[:], rb[:],
                                         start=True, stop=True)
                        bc1 = a1.tile([P, TW], f32, tag="bc1", bufs=1, name="bc1")
                        nc.vector.tensor_copy(out=bc1[:], in_=bcp[:])

                        # q + k_pe projections: copy raw to SBUF immediately
                        # (frees PSUM), scale by r1 in place once bc1 is ready
                        for f in range(3):
                            qp = aps.tile([P, TW], f32, tag="big", bufs=2, name="qp")
                            for o in range(HCH):
                                nc.tensor.matmul(qp[:], wq_sb[:, o, f * P:(f + 1) * P],
                                                 ht_t[:, o, :],
                                                 start=(o == 0), stop=(o == HCH - 1))
                            nc.vector.tensor_copy(
                                out=qsb[:, f, t * TW:(t + 1) * TW], in_=qp[:])
                        kp2 = aps.tile([P, TW], f32, tag="big", bufs=2, name="kp2")
                        for o in range(HCH):
                            nc.tensor.matmul(kp2[:DR, :],
                                             wkva_sb[:, o, KVR:KVR + DR],
                                             ht_t[:, o, :],
                                             start=(o == 0), stop=(o == HCH - 1))
                        nc.vector.tensor_copy(out=kpe2[:DR, t * TW:(t + 1) * TW],
                                              in_=kp2[:DR, :])
                        for f in range(3):
                            nc.vector.tensor_tensor(qsb[:, f, t * TW:(t + 1) * TW],
                                                    qsb[:, f, t * TW:(t + 1) * TW],
                                                    bc1[:], MUL)
                        nc.vector.tensor_tensor(kpe2[:DR, t * TW:(t + 1) * TW],
                                                kpe2[:DR, t * TW:(t + 1) * TW],
                                                bc1[:DR, :], MUL)
                        # duplicate rope rows for the second attention head
                        nc.sync.dma_start(kpe2[DR:, t * TW:(t + 1) * TW],
                                          kpe2[:DR, t * TW:(t + 1) * TW])

                # ---- A2: rope, kv_b, attention, o_proj (+AR) ----
                with tc.tile_pool(name="a2", bufs=1) as a2:
                    cs = a2.tile([P, T], bf16)
                    nc.scalar.dma_start(cs[:], cosf.ap())
                    sn = a2.tile([P, T], bf16)
                    nc.scalar.dma_start(sn[:], sinf.ap())
                    msk = a2.tile([P, 4, TW], f32)
                    nc.scalar.dma_start(msk[:], masks.ap())
                    # prefetch the big MLP weights early (consumed in phase B)
                    wg_sb = mlpw.tile([P, HCH, FPC], bf16)       # 32K
                    nc.scalar.dma_start(wg_sb[:], wg.ap())
                    wu_sb = mlpw.tile([P, HCH, FPC], bf16)       # 32K
                    nc.scalar.dma_start(wu_sb[:], wu.ap())

                    # kv_b: k_nope (transposed out) + v (natural out)
                    knope = a2.tile([P, HPC, T], bf16)
                    for h in range(HPC):
                        for t2 in range(NC_N):
                            kp = aps.tile([P, TO], f32, tag="big", bufs=2, name="kp")
                            for c in range(KC):
                                nc.tensor.matmul(kp[:],
                                                 wkvb_sb[:, c, h * P:(h + 1) * P],
                                                 kva2[:, t2, c, :],
                                                 start=(c == 0), stop=(c == KC - 1))
                            nc.vector.tensor_copy(
                                out=knope[:, h, t2 * TO:(t2 + 1) * TO], in_=kp[:])
                    vnat = a2.tile([P, T // P, HPC * DV], bf16)
                    for to in range(T // P):
                        vp = aps.tile([P, HPC * DV], f32, tag="vp", bufs=1, name="vp")
                        for c in range(KC):
                            nc.tensor.matmul(vp[:],
                                             kva2[:, to // 2, c,
                                                  (to % 2) * P:(to % 2 + 1) * P],
                                             wkvb_sb[:, c, HPC * DN:],
                                             start=(c == 0), stop=(c == KC - 1))
                        nc.vector.tensor_copy(out=vnat[:, to, :], in_=vp[:])

                    # rope in place: qsb[:,2,:] rows are [h0x1,h0x2,h1x1,h1x2],
                    # kpe2 rows are [x1,x2,x1,x2]; cs=[c,c,c,c], sn=[-s,s,-s,s]
                    for src in (qsb[:, 2, :], kpe2[:]):
                        swp = a2.tile([P, T], bf16, tag="swp", bufs=2, name="swp")
                        for g in range(4):
                            half = 32 if g % 2 == 0 else -32
                            nc.sync.dma_start(swp[g * 32:(g + 1) * 32, :],
                                              src[g * 32 + half:(g + 1) * 32 + half, :])
                        rtmp = a2.tile([P, T], bf16, tag="rtmp", bufs=2, name="rtmp")
                        nc.vector.tensor_tensor(rtmp[:], src, cs[:], MUL)
                        nc.vector.tensor_tensor(src, swp[:], sn[:], MUL)
                        nc.vector.tensor_tensor(src, src, rtmp[:], ADD)

                    # attention (scores transposed: [k, q]) + o_proj partial + AR
                    for b in range(B):
                        for qt in range(QT):
                            tt = b * QT + qt
                            qc0 = b * S + qt * TW
                            nkt = 4 * qt + 4
                            attn_t = a2.tile([P, HPC, TW], bf16, tag="attn",
                                             bufs=2, name="attn_t")
                            for h in range(HPC):
                                dnp = aps.tile([1, TW], f32, tag="den", bufs=1,
                                               name="dnp")
                                atp = aps.tile([P, TW], f32, tag="att", bufs=2,
                                               name="atp")
                                exs = [None] * nkt

                                def consume(kt):
                                    nc.tensor.matmul(dnp[:], ones_col[:], exs[kt][:],
                                                     start=(kt == 0),
                                                     stop=(kt == nkt - 1))
                                    nc.tensor.matmul(atp[:],
                                                     vnat[:, b * KT + kt,
                                                          h * DV:(h + 1) * DV],
                                                     exs[kt][:],
                                                     start=(kt == 0),
                                                     stop=(kt == nkt - 1))

                                for kt in range(nkt):
                                    kc0 = b * S + kt * P
                                    scp = aps.tile([P, TW], f32, tag="big", bufs=2,
                                                   name="scp")
                                    nc.tensor.matmul(scp[:],
                                                     knope[:, h, kc0:kc0 + P],
                                                     qsb[:, h, qc0:qc0 + TW],
                                                     start=True, stop=False)
                                    nc.tensor.matmul(
                                        scp[:],
                                        kpe2[h * DR:(h + 1) * DR, kc0:kc0 + P],
                                        qsb[h * DR:(h + 1) * DR, 2, qc0:qc0 + TW],
                                        start=False, stop=True)
                                    ex = awrk.tile([P, TW], bf16, tag="ex", bufs=4,
                                                   name="ex")
                                    j = kt - 4 * qt
                                    if j >= 0:
                                        mtmp = awrk.tile([P, TW], f32, tag="mt",
                                                         name="mtmp")
                                        nc.vector.tensor_tensor(mtmp[:], scp[:],
                                                                msk[:, j, :], ADD)
                                        nc.scalar.activation(ex[:], mtmp[:], AF.Exp)
                                    else:
                                        nc.scalar.activation(ex[:], scp[:], AF.Exp)
                                    exs[kt] = ex
                                    if kt >= 2:
                                        consume(kt - 2)
                                consume(max(nkt - 2, 0))
                                if nkt > 1:
                                    consume(nkt - 1)
                                drow = arow.tile([1, TW], bf16, tag="rb", name="drow")
                                with nc.allow_low_precision(reason="softmax denom"):
                                    nc.vector.reciprocal(drow[:], dnp[:])
                                dbp = aps.tile([P, TW], f32, tag="big", bufs=2,
                                               name="dbp")
                                nc.tensor.matmul(dbp[:], ones_row[:], drow[:],
                                                 start=True, stop=True)
                                dbc = awrk.tile([P, TW], f32, tag="mt", name="dbc")
                                nc.vector.tensor_copy(out=dbc[:], in_=dbp[:])
                                nc.vector.tensor_tensor(
                                    attn_t[:, h, :], atp[:], dbc[:], MUL)
                            # o_proj partial for this token chunk
                            oall = a2.tile([P, HCH, TW], bf16, tag="oall", bufs=1,
                                           name="oall")
                            for ho in range(HCH):
                                op = aps.tile([P, TW], f32, tag="big", bufs=2,
                                              name="op")
                                for h in range(HPC):
                                    nc.tensor.matmul(op[:],
                                                     wo_sb[:, h, ho * P:(ho + 1) * P],
                                                     attn_t[:, h, :],
                                                     start=(h == 0),
                                                     stop=(h == HPC - 1))
                                nc.vector.tensor_copy(out=oall[:, ho, :], in_=op[:])
                            nc.sync.dma_start(ar_in[tt][:], oall[:])
                            nc.gpsimd.collective_compute(
                                "ReduceScatter", ADD, ins=[ar_in[tt][:].opt()],
                                outs=[o_rs[tt][:].opt()], replica_groups=rg)
                            nc.gpsimd.collective_compute(
                                "AllGather", BYP, ins=[o_rs[tt][:].opt()],
                                outs=[o_ag[tt][:].opt()], replica_groups=rg)

            # ============ Phase B: residual + norm + MLP ============
            with tc.tile_pool(name="bbig", bufs=1) as bbig, \
                 tc.tile_pool(name="bwrk", bufs=2) as bwrk, \
                 tc.tile_pool(name="brow", bufs=1) as brow, \
                 tc.tile_pool(name="bps", bufs=1, space="PSUM") as bps:

                wd_sb = bbig.tile([P, FPC // P, HID], bf16)  # 32K
                nc.scalar.dma_start(wd_sb[:], wd.ap())

                for t in range(TT):
                    # x = hidden + attn_out; later x/8 in place
                    x = bbig.tile([P, HCH, TW], bf16, name="x", tag="x", bufs=2)
                    nc.sync.dma_start(x[:], htb.ap()[:, t, :, :])
                    arall = bbig.tile([P, HCH, TW], bf16, name="arall", tag="ar",
                                      bufs=1)
                    nc.sync.dma_start(arall[:], o_ag[t][:])
                    ssp3 = bps.tile([1, TW], f32, tag="ss", bufs=1, name="ssp3")
                    for g in range(4):
                        sqg3 = bwrk.tile([P, TW], bf16, tag="sqg3", bufs=2,
                                         name="sqg3")
                        for k in range(4):
                            o = 4 * g + k
                            nc.vector.tensor_tensor(x[:, o, :], x[:, o, :],
                                                    arall[:, o, :], ADD)
                            if k == 0:
                                nc.scalar.square(sqg3[:], x[:, o, :])
                            else:
                                sq3 = bwrk.tile([P, TW], bf16, tag="sq3", bufs=2,
                                                name="sq3")
                                nc.scalar.square(sq3[:], x[:, o, :])
                                nc.vector.tensor_tensor(sqg3[:], sqg3[:], sq3[:],
                                                        ADD)
                        nc.tensor.matmul(ssp3[:], ones_col[:], sqg3[:],
                                         start=(g == 0), stop=(g == 3))
                    srow3 = brow.tile([1, TW], f32, tag="srow3", name="srow3")
                    nc.scalar.activation(srow3[:], ssp3[:], AF.Sqrt,
                                         bias=epsb[:], scale=1.0 / HID)
                    rrow3 = brow.tile([1, TW], f32, tag="rrow3", name="rrow3")
                    nc.vector.reciprocal(rrow3[:], srow3[:])
                    rb3 = brow.tile([1, TW], bf16, tag="rb3", name="rb3")
                    nc.vector.tensor_copy(out=rb3[:], in_=rrow3[:])
                    bcp3 = bps.tile([P, TW], f32, tag="gu", bufs=4, name="bcp3")
                    nc.tensor.matmul(bcp3[:], ones_row[:], rb3[:], start=True,
                                     stop=True)
                    bc3 = bwrk.tile([P, TW], f32, tag="bc3", bufs=1, name="bc3")
                    nc.vector.tensor_copy(out=bc3[:], in_=bcp3[:])
                    h2 = bbig.tile([P, HCH, TW], bf16, name="h2", tag="h2", bufs=1)
                    for o in range(HCH):
                        nc.vector.tensor_tensor(h2[:, o, :], x[:, o, :], bc3[:], MUL)
                    # x -> x/8 in place (folded residual for ReduceScatter)
                    nc.vector.tensor_scalar_mul(x[:, :, :], x[:, :, :], 0.125)

                    # gate/up/silu
                    act = bbig.tile([P, FPC // P, TW], bf16, name="act", tag="act",
                                    bufs=1)
                    for fi in range(FPC // P):
                        gp = bps.tile([P, TW], f32, tag="gu", bufs=4, name="gp")
                        for o in range(HCH):
                            nc.tensor.matmul(gp[:], wg_sb[:, o, fi * P:(fi + 1) * P],
                                             h2[:, o, :],
                                             start=(o == 0), stop=(o == HCH - 1))
                        up = bps.tile([P, TW], f32, tag="gu", bufs=4, name="up")
                        for o in range(HCH):
                            nc.tensor.matmul(up[:], wu_sb[:, o, fi * P:(fi + 1) * P],
                                             h2[:, o, :],
                                             start=(o == 0), stop=(o == HCH - 1))
                        gs = bwrk.tile([P, TW], f32, tag="gs", bufs=2, name="gs")
                        nc.scalar.activation(gs[:], gp[:], AF.Silu)
                        nc.vector.tensor_tensor(act[:, fi, :], up[:], gs[:], MUL)

                    # down projection partial (+x/8) + RS
                    # last chunk reduces in two 1MB halves to shrink the tail
                    dall = bbig.tile([P, HCH, TW], bf16, name="dall", tag="dall",
                                     bufs=1)
                    for ho in range(HCH):
                        dpp = bps.tile([P, TW], f32, tag="d", bufs=2, name="dpp")
                        for c in range(FPC // P):
                            nc.tensor.matmul(dpp[:], wd_sb[:, c, ho * P:(ho + 1) * P],
                                             act[:, c, :],
                                             start=(c == 0), stop=(c == FPC // P - 1))
                        nc.vector.tensor_tensor(dall[:, ho, :], dpp[:], x[:, ho, :],
                                                ADD)
                        if t == TT - 1 and ho == HCH // 2 - 1:
                            nc.sync.dma_start(rs_in[3][:], dall[:, :HCH // 2, :])
                            nc.gpsimd.collective_compute(
                                "ReduceScatter", ADD, ins=[rs_in[3][:].opt()],
                                outs=[rs_out[3][:].opt()], replica_groups=rg)
                            nc.sync.dma_start(
                                out.ap()[:, 3 * HCH * TW:
                                         3 * HCH * TW + HCH // 2 * TW],
                                rs_out[3][:])
                    if t < TT - 1:
                        nc.sync.dma_start(rs_in[t][:], dall[:])
                        nc.gpsimd.collective_compute(
                            "ReduceScatter", ADD, ins=[rs_in[t][:].opt()],
                            outs=[rs_out[t][:].opt()], replica_groups=rg)
                        nc.sync.dma_start(
                            out.ap()[:, t * HCH * TW:(t + 1) * HCH * TW],
                            rs_out[t][:])
                    else:
                        nc.sync.dma_start(rs_in[4][:], dall[:, HCH // 2:, :])
                        nc.gpsimd.collective_compute(
                            "ReduceScatter", ADD, ins=[rs_in[4][:].opt()],
                            outs=[rs_out[4][:].opt()], replica_groups=rg)
                        nc.sync.dma_start(
                            out.ap()[:, 3 * HCH * TW + HCH // 2 * TW:],
                            rs_out[4][:])
    nc.compile()
    return nc


def _prep(hidden_states, positions, w_in_ln, w_q, w_kv_a, w_kv_a_ln,
          w_kv_b, w_o, w_post_ln, w_gate, w_up, w_down):
    hT = np.ascontiguousarray(
        np.asarray(hidden_states, np.float32).reshape(T, HID).T)
    hTt = hT.reshape(HCH, P, T).transpose(1, 0, 2)          # [128, 16, T]
    htb = np.ascontiguousarray(
        hTt.reshape(P, HCH, TT, TW).transpose(0, 2, 1, 3)).astype(BF)

    pos = np.asarray(positions).reshape(-1).astype(np.float64)
    inv = ROPE_BASE ** (-np.arange(0, DR, 2, dtype=np.float64) / DR)
    fr = pos[:, None] * inv[None, :]                      # [T, 32]
    c32 = np.cos(fr).T.astype(np.float32)                 # [32, T]
    s32 = np.sin(fr).T.astype(np.float32)
    cosf = np.concatenate([c32] * 4, 0)
    sinf = np.concatenate([-s32, s32, -s32, s32], 0)

    r = np.arange(P)[:, None]
    c = np.arange(TW)[None, :]
    masks = np.stack([np.where(c >= r + j * P, 0.0, NEG) for j in range(4)],
                     1).astype(np.float32)                # [128, 4, 512]

    def tilemaj(a, chunks):
        # [rows, cols] -> [128, chunks, cols], partition-major
        return np.ascontiguousarray(
            a.reshape(chunks, P, -1).transpose(1, 0, 2)).astype(BF)

    w_in_ln = np.asarray(w_in_ln, np.float32)
    wqf = (np.asarray(w_q, np.float32) * w_in_ln[:, None] * SCALING
           ).reshape(HID, H, DQK)
    wkvaf = np.asarray(w_kv_a, np.float32) * w_in_ln[:, None]
    kpe_w = wkvaf[:, KVR:]
    pe_pair = np.concatenate([kpe_w[:, 0::2], kpe_w[:, 1::2]], 1)
    wkva_p = np.concatenate([wkvaf[:, :KVR], pe_pair], 1)
    wkvbf = (np.asarray(w_kv_b, np.float32)
             * np.asarray(w_kv_a_ln, np.float32)[:, None]).reshape(KVR, H, DN + DV)
    w_post_ln = np.asarray(w_post_ln, np.float32)
    wgf = np.asarray(w_gate, np.float32) * w_post_ln[:, None]
    wuf = np.asarray(w_up, np.float32) * w_post_ln[:, None]
    wdf = np.asarray(w_down, np.float32)
    wof = np.asarray(w_o, np.float32).reshape(H, DV, HID)

    in_maps = []
    for core in range(NC_N):
        hs = [2 * core, 2 * core + 1]
        nopes = np.concatenate([wqf[:, h, :DN] for h in hs], 1)
        pes = []
        for h in hs:
            pe = wqf[:, h, DN:]
            pes += [pe[:, 0::2], pe[:, 1::2]]
        wq_c = np.concatenate([nopes] + pes, 1)
        wkvb_c = np.concatenate(
            [wkvbf[:, hs[0], :DN], wkvbf[:, hs[1], :DN],
             wkvbf[:, hs[0], DN:], wkvbf[:, hs[1], DN:]], 1)   # [512, 512]
        in_maps.append({
            "htb": htb,
            "hto": np.ascontiguousarray(
                hTt[:, :, core * TO:(core + 1) * TO]).astype(BF),
            "wq": tilemaj(wq_c, HCH),
            "wkva": tilemaj(wkva_p, HCH),
            "wkvb": tilemaj(wkvb_c, KC),
            "wo": np.ascontiguousarray(
                np.concatenate([wof[h] for h in hs], 0).reshape(
                    HPC, P, HID).transpose(1, 0, 2)).astype(BF),
            "wg": tilemaj(wgf[:, core * FPC:(core + 1) * FPC], HCH),
            "wu": tilemaj(wuf[:, core * FPC:(core + 1) * FPC], HCH),
            "wd": tilemaj(wdf[core * FPC:(core + 1) * FPC, :], FPC // P),
            "cosf": cosf.astype(BF),
            "sinf": sinf.astype(BF),
            "masks": masks,
        })
    return in_maps


def kernel(**inputs):
    if "nc" not in _CACHE:
        _CACHE["nc"] = _build()
    nc = _CACHE["nc"]
    in_maps = _prep(**inputs)
    res = run_bass_kernel_spmd(nc, in_maps, core_ids=list(range(NC_N)))
    xT = np.empty((HCH, NC_N, 16, T), np.float32)
    for c in range(NC_N):
        slab = np.asarray(res.results[c]["o"], dtype=np.float32)
        slab = slab.reshape(16, TT, HCH, TW)          # [p, t, o, col]
        xT[:, c, :, :] = slab.transpose(2, 0, 1, 3).reshape(HCH, 16, T)
    return np.ascontiguousarray(
        xT.reshape(HID, T).T).reshape(B, S, HID)
